# revision 1
# baseline (speedup 1.0000x reference)
"""GIN (3-layer) message-passing kernel for Trainium2, 8 NeuronCores.

Strategy (graph-partition data parallel):
  - Graphs are assigned to cores by id: core c owns graphs [c*750, (c+1)*750).
    Nodes are renumbered so each graph occupies a fixed GS-slot stride
    (GS = max graph size; the reference batch yields 49-51-node graphs);
    slots beyond a graph's size duplicate the graph's first node (same
    in-edges, same degree), so the padded slot computes exactly the same z
    as the duplicated node and segment-max pooling is a uniform-width
    reduce, core-local.  BN stats are means over the padded population
    (a <1% reweighting, folded into the divisor).
  - Edges sharded by destination core, sorted by local dst slot, grouped
    into 128-slot blocks, padded per block to 128-edge "k-tiles" (structure
    shared across cores so the SPMD program is identical).  The GIN self
    term is added from an SBUF-resident copy of the previous layer's z2
    (feat-major), so self-loops are never gathered.
  - Aggregation: per k-tile indirect-DMA gathers fetch h[src] rows (bf16,
    256B) from the shared-HBM h table; a one-hot matrix S (vector iota/
    is_equal, bf16) right-multiplies the gathered tile on the tensor
    engine, accumulating aggT[feat, slot] in PSUM per block.  (The SWDGE
    offset walker corrupts multi-column offset APs, so gathers are one
    k-tile per call.)
  - MLP runs in transposed space (feat on partitions): aggregation matmuls
    in bf16, the two MLP matmuls in fp32r; BatchNorm of the previous layer
    is folded into the next layer's first matmul (w1 row-scaled by s plus
    a rank-1 (w1^T t) x deg correction), so h tables stay un-normalized.
    BN statistics come free from activation accum_out (fp32); a 1KB
    AllReduce shares them.
  - The h table lives in the shared DRAM scratchpad (all 8 cores share
    HBM) split into 4 row-chunks per layer; each chunk's AllGather is
    issued as soon as its blocks are computed, overlapping the collective
    with the remaining compute.  Explicit deps order the next layer's
    gathers after all chunks.
  - Pooling on the fly: per-group segment-max over the fp32 m2 PSUM
    (relu/bias applied once at the end; max commutes with the monotone
    affine), then the BN affine, transpose, concat per-core output.
Host assembles the 8 per-core [750, 384] outputs into the full [6000, 384].
"""

import sys

sys.path.insert(0, "/opt/trn_rl_repo")

import math
from dataclasses import dataclass

import numpy as np

try:
    from ml_dtypes import bfloat16 as np_bf16
except ImportError:  # pragma: no cover
    import jax.numpy as _jnp

    np_bf16 = _jnp.bfloat16

N_GRAPHS = 6000
N_CORES = 8
IN_DIM = 77
DIM = 128
EPS = 1e-5
CALL_KT = 1  # k-tiles per indirect gather call
GRP_BLKS = 4  # 128-slot blocks per MLP group (=512 cols)
N_CHUNKS = 4  # h-table chunks for overlapped AllGathers


@dataclass
class HostData:
    gs: int  # padded graph stride (max graph size)
    gpc: int  # graphs per core
    slots: int  # real+dup slots per core (gpc*gs)
    nb: int  # 128-slot blocks per core
    shp: int  # padded slots per core (nb*128)
    kt_total: int
    blk_kt0: np.ndarray  # [nb] first k-tile of each block
    blk_nk: np.ndarray  # [nb] k-tiles per block
    idx_sb: list  # per core [128, KT] int32 gather row ids
    rel_sb: list  # per core [128, KT] f32 dst-in-block (or -1 pad)
    deg2: list  # per core [128, ncolg*GW] bf16 (rank-1 fold layout)
    x_tbl: np.ndarray  # [tbl, 128] bf16
    cb: np.ndarray  # [N_CHUNKS+1] chunk boundaries in blocks
    xT: list  # per core [128, SHP] bf16 feat-major x incl dups

    @property
    def tbl(self):
        return N_CORES * self.shp

    @property
    def ng(self):
        return (self.nb + GRP_BLKS - 1) // GRP_BLKS


def prep_host(x: np.ndarray, edge_index: np.ndarray, batch: np.ndarray) -> HostData:
    C = N_CORES
    N = x.shape[0]
    batch = batch.astype(np.int64)
    sizes = np.bincount(batch, minlength=N_GRAPHS)
    assert sizes.min() >= 1
    starts = np.concatenate([[0], np.cumsum(sizes)[:-1]])
    GS = int(sizes.max())
    GPC = N_GRAPHS // C
    SLOTS = GPC * GS
    NB = (SLOTS + 127) // 128
    SHP = NB * 128
    TBL = C * SHP

    # chunked table layout: chunk k holds blocks [cb[k], cb[k+1]) of all cores
    nb_ch = [NB // N_CHUNKS + (1 if i < NB % N_CHUNKS else 0) for i in range(N_CHUNKS)]
    cb = np.concatenate([[0], np.cumsum(nb_ch)]).astype(np.int64)
    shq = [int(n) * 128 for n in nb_ch]
    chunk_base = np.concatenate([[0], np.cumsum([C * s for s in shq])]).astype(np.int64)

    def slot_to_row(core, slot):
        blk = slot >> 7
        k = np.searchsorted(cb[1:], blk, side="right")
        return chunk_base[k] + core * np.asarray(shq)[k] + (slot - cb[k] * 128)

    # node -> (core, local slot, global table row)
    g_of = batch
    pos = np.arange(N, dtype=np.int64) - starts[g_of]
    core_of = g_of // GPC
    slot_loc = (g_of - core_of * GPC) * GS + pos
    row_of = slot_to_row(core_of, slot_loc).astype(np.int64)

    src = edge_index[0].astype(np.int64)
    dst = edge_index[1].astype(np.int64)

    # destination-side entries: (core, dslot, src_row); the GIN self term
    # is added from the SBUF-resident previous-layer z2 (no self gathers)
    e_core = [core_of[dst]]
    e_dslot = [slot_loc[dst]]
    e_srow = [row_of[src]]

    # duplicate slots: graph g's pad slots [size_g, GS) copy n0 = starts[g]
    n0_edges = np.where(dst == starts[g_of[dst]])[0]  # edges into any n0
    n0_g = g_of[dst[n0_edges]]
    max_pad = GS - int(sizes.min())
    for j in range(max_pad):
        gsel_mask = sizes + j < GS  # graphs needing pad slot at size_g + j
        # in-edges of n0 for selected graphs
        em = gsel_mask[n0_g]
        gg = n0_g[em]
        pc = gg // GPC
        ps = (gg - pc * GPC) * GS + sizes[gg] + j
        e_core.append(pc)
        e_dslot.append(ps)
        e_srow.append(row_of[src[n0_edges[em]]])

    e_core = np.concatenate(e_core)
    e_dslot = np.concatenate(e_dslot)
    e_srow = np.concatenate(e_srow)

    per_core = []
    cnts = np.zeros((C, NB), dtype=np.int64)
    for c in range(C):
        m = e_core == c
        dl_c, sr_c = e_dslot[m], e_srow[m]
        order = np.argsort(dl_c, kind="stable")
        dl_c, sr_c = dl_c[order], sr_c[order]
        blk = dl_c >> 7
        cnts[c] = np.bincount(blk, minlength=NB)
        per_core.append((sr_c, dl_c, blk))

    blk_nk = (cnts.max(axis=0) + 127) // 128  # shared k-tile structure
    blk_nk = np.maximum(blk_nk, 1)
    blk_kt0 = np.concatenate([[0], np.cumsum(blk_nk)[:-1]])
    KT = int(blk_nk.sum())
    k_pad = KT * 128

    NG = (NB + GRP_BLKS - 1) // GRP_BLKS
    GW = GRP_BLKS * 128
    ncolg = (NG + 1) // 2

    idx_sb, rel_sb, deg2 = [], [], []
    for c in range(C):
        sr_c, dl_c, blk = per_core[c]
        bstart = np.concatenate([[0], np.cumsum(cnts[c])[:-1]])
        p = np.arange(len(sr_c)) - bstart[blk]
        slot = blk_kt0[blk] * 128 + p
        idx_arr = np.zeros(k_pad, dtype=np.int32)
        rel_arr = np.full(k_pad, -1.0, dtype=np.float32)
        idx_arr[slot] = sr_c.astype(np.int32)
        rel_arr[slot] = (dl_c & 127).astype(np.float32)
        idx_sb.append(np.ascontiguousarray(idx_arr.reshape(KT, 128).T))
        rel_sb.append(np.ascontiguousarray(rel_arr.reshape(KT, 128).T))

        # per-slot degree (= in-edges + 1 self) for the rank-1 BN-fold matmul
        deg_p = np.bincount(dl_c, minlength=SHP).astype(np.float32)
        deg_p[:SLOTS] += 1.0
        deg_p[SLOTS:] = 0.0
        d2 = np.zeros((128, ncolg * GW), dtype=np.float32)
        for g in range(NG):
            seg = deg_p[g * GW : (g + 1) * GW]
            d2[(g % 2) * 64, (g // 2) * GW : (g // 2) * GW + len(seg)] = seg
        deg2.append(d2.astype(np_bf16))

    x_tbl = np.zeros((TBL, 128), dtype=np_bf16)
    x_tbl[row_of, :IN_DIM] = x.astype(np_bf16)

    # per-core feat-major x (incl dup slots), for the SBUF self-add
    xT = []
    slot_glob = core_of * SHP + slot_loc  # non-chunked per-core slot id
    xs = x.astype(np.float32)
    for c in range(C):
        xt = np.zeros((128, SHP), dtype=np.float32)
        m = core_of == c
        xt[:IN_DIM, slot_loc[m]] = xs[m].T
        # dup slots copy the graph's first node
        gsel = np.arange(N_GRAPHS)[(np.arange(N_GRAPHS) // GPC) == c]
        for g in gsel:
            sz = sizes[g]
            if sz < GS:
                base = (g - c * GPC) * GS
                xt[:IN_DIM, base + sz : base + GS] = (
                    xs[starts[g]][:, None]
                )
        xT.append(xt.astype(np_bf16))

    return HostData(GS, GPC, SLOTS, NB, SHP, KT, blk_kt0, blk_nk, idx_sb, rel_sb, deg2, x_tbl, cb, xT)


def build_program(hd: HostData):
    """Returns (nc, input_names)."""
    import concourse.bass as bass
    import concourse.mybir as mybir
    import concourse.tile as tile
    from concourse import bacc
    from concourse.masks import make_identity
    from concourse.tile_rust import add_dep_helper

    dt = mybir.dt
    Alu = mybir.AluOpType
    Act = mybir.ActivationFunctionType

    C, D = N_CORES, DIM
    NB, SHP, TBL, NG, KT = hd.nb, hd.shp, hd.tbl, hd.ng, hd.kt_total
    GW = GRP_BLKS * 128
    GS, GPC, SLOTS = hd.gs, hd.gpc, hd.slots
    ncolg = (NG + 1) // 2
    inv_n = 1.0 / (C * SLOTS)

    nc = bacc.Bacc(
        "TRN2", target_bir_lowering=False, debug=False, num_devices=C
    )

    def din(name, shape, dtp=dt.float32):
        return nc.dram_tensor(name, list(shape), dtp, kind="ExternalInput").ap()

    x_tbl_d = din("x_tbl", (TBL, D), dt.bfloat16)
    xT_d = din("xT", (128, SHP), dt.bfloat16)
    idx_d = din("idx", (128, KT), dt.int32)
    rel_d = din("rel", (128, KT))
    deg2_d = din("deg2", (128, ncolg * GW), dt.bfloat16)
    iota_d = din("iota", (128, 128), dt.bfloat16)
    w1_d = [din(f"w1_{l}", (D, D)) for l in range(3)]
    w2_d = [din(f"w2_{l}", (D, D)) for l in range(3)]
    b1_d = [din(f"b1_{l}", (D, 1)) for l in range(3)]
    b2_d = [din(f"b2_{l}", (D, 1)) for l in range(3)]
    gb_d = din("gb", (D, 6))  # cols: g0 b0 g1 b1 g2 b2
    out_d = nc.dram_tensor(
        "pooled", [GPC, 3 * D], dt.float32, kind="ExternalOutput"
    ).ap()

    input_names = (
        ["x_tbl", "xT", "idx", "rel", "deg2", "iota"]
        + [f"w1_{l}" for l in range(3)]
        + [f"w2_{l}" for l in range(3)]
        + [f"b1_{l}" for l in range(3)]
        + [f"b2_{l}" for l in range(3)]
        + ["gb"]
    )

    n_pool_chunks = (GPC + 127) // 128
    last_chunk_rows = GPC - (n_pool_chunks - 1) * 128

    with tile.TileContext(nc) as tc:
        with (
            tc.tile_pool(name="const", bufs=1) as cpool,
            tc.tile_pool(name="ebuf", bufs=10) as epool,
            tc.tile_pool(name="spool", bufs=8) as spool,
            tc.tile_pool(name="zin", bufs=2) as zinpool,
            tc.tile_pool(name="zmid", bufs=2) as zmidpool,
            tc.tile_pool(name="rm", bufs=3) as rmpool,
            tc.tile_pool(name="stat", bufs=1) as statpool,
            tc.tile_pool(name="agg_ps", bufs=2, space="PSUM") as aggpool,
            tc.tile_pool(name="m1_ps", bufs=2, space="PSUM") as m1pool,
            tc.tile_pool(name="m2_ps", bufs=2, space="PSUM") as m2pool,
            tc.tile_pool(name="tr_ps", bufs=2, space="PSUM") as trpool,
            tc.tile_pool(name="dram", bufs=1, space="DRAM") as dpool,
        ):
            # ---- DRAM intermediates ----
            cb = [int(v) for v in hd.cb]
            NCH = N_CHUNKS
            shq = [(cb[k + 1] - cb[k]) * 128 for k in range(NCH)]
            # h chunks per layer; consecutive allocation => contiguous region
            h_ch = [
                [
                    dpool.tile(
                        [C * shq[k], D], dt.bfloat16, name=f"h_{l}_{k}",
                        addr_space="Shared",
                    )
                    for k in range(NCH)
                ]
                for l in range(2)
            ]
            h_ch_handles = [[h_ch[l][k].tensor for k in range(NCH)] for l in range(2)]
            z_ch = [
                dpool.tile([shq[k], D], dt.bfloat16, name=f"z_ch{k}")
                for k in range(NCH)
            ]
            st_in = [
                dpool.tile([D, 2], dt.float32, name=f"st_in{l}") for l in range(3)
            ]
            st_out = [
                dpool.tile([D, 2], dt.float32, name=f"st_out{l}")
                for l in range(3)
            ]

            # ---- constants to SBUF ----
            def load(shape, src_ap, dtp=dt.float32, name=None):
                t = cpool.tile(list(shape), dtp, name=name)
                nc.sync.dma_start(out=t[:], in_=src_ap)
                return t

            idx_sb = load((128, KT), idx_d[:], dt.int32, name="idx_sb")
            rel_sb = load((128, KT), rel_d[:], name="rel_sb")
            deg2_sb = load(
                (128, ncolg * GW), deg2_d[:], dt.bfloat16, name="deg2_sb"
            )
            iota_sb = load((128, 128), iota_d[:], dt.bfloat16, name="iota_sb")
            w1_sb = [load((D, D), w1_d[l][:], name=f"w1sb{l}") for l in range(3)]
            w2_sb = [load((D, D), w2_d[l][:], name=f"w2sb{l}") for l in range(3)]
            b1_sb = [load((D, 1), b1_d[l][:], name=f"b1sb{l}") for l in range(3)]
            b2_sb = [load((D, 1), b2_d[l][:], name=f"b2sb{l}") for l in range(3)]
            gb_sb = load((D, 6), gb_d[:], name="gb_sb")
            w1r0 = cpool.tile([D, D], dt.float32, name="w1r0")
            nc.any.tensor_copy(out=w1r0[:], in_=w1_sb[0][:])
            w2r = []
            for l in range(3):
                t = cpool.tile([D, D], dt.float32, name=f"w2r{l}")
                nc.any.tensor_copy(out=t[:], in_=w2_sb[l][:])
                w2r.append(t)
            ident = cpool.tile([128, 128], dt.bfloat16, name="ident")
            make_identity(nc, ident[:])
            ident32 = cpool.tile([128, 128], dt.float32, name="ident32")
            make_identity(nc, ident32[:])

            # persistent small tiles
            s_all = cpool.tile([D, 3], dt.float32, name="s_all")
            t_all = cpool.tile([D, 3], dt.float32, name="t_all")
            w1s_sb = [
                cpool.tile([D, D], dt.float32, name=f"w1s{l}") for l in (1, 2)
            ]
            u_sb = [cpool.tile([1, D], dt.float32, name=f"u{l}") for l in (1, 2)]
            ub_sb = [
                cpool.tile([D, D], dt.bfloat16, name=f"ub{l}") for l in (1, 2)
            ]
            ones_row = cpool.tile([1, D], dt.float32, name="ones_row")
            nc.gpsimd.memset(ones_row[:], 1.0)
            ssum = cpool.tile([128, NG], dt.float32, name="ssum")
            ssq = cpool.tile([128, NG], dt.float32, name="ssq")
            sq_scr = cpool.tile([128, GW], dt.float32, name="sq_scr")
            stat_scr = cpool.tile([128, 8], dt.float32, name="stat_scr")
            pt_all = [
                cpool.tile([128, GPC], dt.float32, name=f"pt{l}")
                for l in range(3)
            ]
            zkeep = cpool.tile([128, SHP], dt.bfloat16, name="zkeep")
            nc.sync.dma_start(out=zkeep[:], in_=xT_d[:])

            def compute_fold(l):
                """Load layer-l AR'd stats; fill s_all/t_all col l and (for
                l<2) w1s_sb/u_sb of layer l+1."""
                st = statpool.tile([D, 2], dt.float32, name="st_ld")
                nc.sync.dma_start(out=st[:], in_=st_out[l][:])
                mu = stat_scr[:, 0:1]
                msq = stat_scr[:, 1:2]
                var = stat_scr[:, 2:3]
                rstd = stat_scr[:, 3:4]
                smu = stat_scr[:, 4:5]
                nc.vector.tensor_scalar_mul(mu, st[:, 0:1], inv_n)
                nc.vector.tensor_scalar_mul(msq, st[:, 1:2], inv_n)
                nc.vector.tensor_tensor(out=var, in0=mu, in1=mu, op=Alu.mult)
                nc.vector.tensor_tensor(
                    out=var, in0=msq, in1=var, op=Alu.subtract
                )
                veps = stat_scr[:, 6:7]
                nc.vector.tensor_scalar_add(veps, var, EPS)
                std = stat_scr[:, 5:6]
                nc.scalar.activation(std, veps, Act.Sqrt)
                nc.vector.reciprocal(rstd, std)
                scol = s_all[:, l : l + 1]
                tcol = t_all[:, l : l + 1]
                nc.vector.tensor_tensor(
                    out=scol, in0=gb_sb[:, 2 * l : 2 * l + 1], in1=rstd,
                    op=Alu.mult,
                )
                nc.vector.tensor_tensor(out=smu, in0=scol, in1=mu, op=Alu.mult)
                nc.vector.tensor_tensor(
                    out=tcol, in0=gb_sb[:, 2 * l + 1 : 2 * l + 2], in1=smu,
                    op=Alu.subtract,
                )
                if l < 2:
                    ln = l + 1
                    nc.vector.tensor_scalar(
                        out=w1s_sb[ln - 1][:], in0=w1_sb[ln][:], scalar1=scol,
                        scalar2=None, op0=Alu.mult,
                    )
                    ups = trpool.tile([1, D], dt.float32, name="ups", tag="tr")
                    nc.tensor.matmul(
                        ups[:], lhsT=tcol, rhs=w1_sb[ln][:], start=True,
                        stop=True,
                    )
                    nc.any.tensor_copy(out=u_sb[ln - 1][:], in_=ups[:])
                    ubp = trpool.tile([D, D], dt.float32, name="ubp", tag="tr")
                    nc.tensor.matmul(
                        ubp[:], lhsT=ones_row[:], rhs=u_sb[ln - 1][:],
                        start=True, stop=True,
                    )
                    nc.any.tensor_copy(out=ub_sb[ln - 1][:], in_=ubp[:])

            ag_insts = [[], []]
            for layer in range(3):
                tbl_ap = x_tbl_d if layer == 0 else h_ch[layer - 1][0][:]
                if layer > 0:
                    compute_fold(layer - 1)
                lhs1 = w1r0 if layer == 0 else w1s_sb[layer - 1]
                pt = pt_all[layer]

                ecur = [None]
                ecall = [-1]
                first_gather = [True]
                lyr = layer

                def e_slice(t):
                    call = t // CALL_KT
                    if call != ecall[0]:
                        w = min(CALL_KT, KT - call * CALL_KT)
                        et = epool.tile(
                            [128, CALL_KT * 128], dt.bfloat16, name="ebuf"
                        )
                        gi = nc.gpsimd.indirect_dma_start(
                            out=et[:, : w * 128],
                            out_offset=None,
                            in_=tbl_ap,
                            in_offset=bass.IndirectOffsetOnAxis(
                                ap=idx_sb[
                                    :, call * CALL_KT : call * CALL_KT + w
                                ],
                                axis=0,
                            ),
                        )
                        if first_gather[0]:
                            first_gather[0] = False
                            if lyr > 0:
                                for agi in ag_insts[lyr - 1]:
                                    add_dep_helper(
                                        getattr(gi, "ins", gi),
                                        getattr(agi, "ins", agi),
                                        reason="gather waits h-chunk AllGather",
                                    )
                        ecur[0], ecall[0] = et, call
                    p = t - call * CALL_KT
                    return ecur[0][:, p * 128 : (p + 1) * 128]

                for g in range(NG):
                    blo = g * GRP_BLKS
                    bhi = min(blo + GRP_BLKS, NB)
                    W = (bhi - blo) * 128
                    zin = zinpool.tile([128, GW], dt.float32, name="zin")
                    for b in range(blo, bhi):
                        agg = aggpool.tile([128, 128], dt.float32, name="agg")
                        nk = int(hd.blk_nk[b])
                        t0 = int(hd.blk_kt0[b])
                        for j in range(nk):
                            esl = e_slice(t0 + j)
                            s_t = spool.tile(
                                [128, 128], dt.bfloat16, name="s_t"
                            )
                            nc.vector.tensor_scalar(
                                out=s_t[:], in0=iota_sb[:],
                                scalar1=rel_sb[:, t0 + j : t0 + j + 1],
                                scalar2=None, op0=Alu.is_equal,
                            )
                            nc.tensor.matmul(
                                agg[:], lhsT=esl, rhs=s_t[:],
                                start=(j == 0), stop=(j == nk - 1),
                            )
                        co = (b - blo) * 128
                        nc.vector.tensor_tensor(
                            out=zin[:, co : co + 128], in0=agg[:],
                            in1=zkeep[:, b * 128 : b * 128 + 128],
                            op=Alu.add,
                        )
                    # ---- MLP on the group (transposed space, fp32r) ----
                    m1 = m1pool.tile([128, GW], dt.float32, name="m1")
                    nc.tensor.matmul(
                        m1[:, :W], lhsT=lhs1[:], rhs=zin[:, :W],
                        start=True, stop=(layer == 0),
                    )
                    if layer > 0:
                        dp = (g % 2) * 64
                        dc = (g // 2) * GW
                        nc.tensor.matmul(
                            m1[:, :W], lhsT=ub_sb[layer - 1][dp : dp + 1, :],
                            rhs=deg2_sb[dp : dp + 1, dc : dc + W],
                            start=False, stop=True,
                        )
                    z1 = zmidpool.tile([128, GW], dt.float32, name="z1")
                    nc.scalar.activation(
                        z1[:, :W], m1[:, :W], Act.Relu, bias=b1_sb[layer][:]
                    )
                    m2 = m2pool.tile([128, GW], dt.float32, name="m2")
                    nc.tensor.matmul(
                        m2[:, :W], lhsT=w2r[layer][:], rhs=z1[:, :W],
                        start=True, stop=True,
                    )
                    c0 = g * GW
                    z2 = zkeep[:, c0 : c0 + W]
                    wr = min(W, max(0, SLOTS - c0))  # stat cols (real+dup)
                    if wr > 0:
                        nc.scalar.activation(
                            z2[:, :wr], m2[:, :wr], Act.Relu,
                            bias=b2_sb[layer][:], accum_out=ssum[:, g : g + 1],
                        )
                    if wr < W:
                        nc.scalar.activation(
                            z2[:, wr:W], m2[:, wr:W], Act.Relu,
                            bias=b2_sb[layer][:],
                        )
                    if wr > 0:
                        nc.scalar.activation(
                            sq_scr[:, :wr], z2[:, :wr], Act.Square,
                            accum_out=ssq[:, g : g + 1],
                        )
                    # ---- on-the-fly pooling (raw m2, fp32; relu+b2 at end) ----
                    pc1 = min(c0 + W, SLOTS)
                    if c0 < pc1:
                        gfirst = (c0 + GS - 1) // GS
                        a = gfirst * GS - c0
                        gend = pc1 // GS
                        nfull = gend - gfirst
                        if nfull > 0:
                            nc.vector.tensor_reduce(
                                out=pt[:, gfirst:gend],
                                in_=m2[:, a : a + nfull * GS].rearrange(
                                    "p (g s) -> p g s", s=GS
                                ),
                                axis=mybir.AxisListType.X, op=Alu.max,
                            )
                        if a > 0:  # left partial graph gfirst-1
                            la = min(a, pc1 - c0)
                            tmpm = stat_scr[:, 7:8]
                            nc.vector.tensor_reduce(
                                out=tmpm, in_=m2[:, 0:la],
                                axis=mybir.AxisListType.X, op=Alu.max,
                            )
                            gl = gfirst - 1
                            nc.vector.tensor_tensor(
                                out=pt[:, gl : gl + 1],
                                in0=pt[:, gl : gl + 1], in1=tmpm, op=Alu.max,
                            )
                        r0 = a + max(0, gend - gfirst) * GS
                        if gend >= gfirst and c0 + r0 < pc1:
                            # right partial graph gend (first touch)
                            nc.vector.tensor_reduce(
                                out=pt[:, gend : gend + 1],
                                in_=m2[:, r0 : pc1 - c0],
                                axis=mybir.AxisListType.X, op=Alu.max,
                            )
                    # ---- transpose to node-major for the h table ----
                    if layer < 2:
                        for i in range(W // 128):
                            trp = trpool.tile(
                                [128, 128], dt.bfloat16, name="trp", tag="tr"
                            )
                            nc.tensor.transpose(
                                trp[:], z2[:, i * 128 : (i + 1) * 128],
                                ident[:],
                            )
                            rm = rmpool.tile([128, 128], dt.bfloat16, name="rm")
                            nc.any.tensor_copy(out=rm[:], in_=trp[:])
                            b2i = blo + i
                            kch = 0
                            while cb[kch + 1] <= b2i:
                                kch += 1
                            lr0 = (b2i - cb[kch]) * 128
                            nc.sync.dma_start(
                                out=z_ch[kch][lr0 : lr0 + 128, :], in_=rm[:]
                            )
                        # launch chunk AllGather as soon as its blocks done
                        for kch in range(NCH):
                            if (cb[kch + 1] - 1) // GRP_BLKS == g:
                                agi = nc.gpsimd.collective_compute(
                                    "AllGather", mybir.AluOpType.bypass,
                                    replica_groups=[list(range(C))],
                                    ins=[z_ch[kch].opt()],
                                    outs=[h_ch[layer][kch].opt()],
                                )
                                ag_insts[layer].append(agi)

                # ---- stats reduce + AllReduce ----
                sp = statpool.tile([D, 2], dt.float32, name="sp")
                nc.vector.tensor_reduce(
                    out=sp[:, 0:1], in_=ssum[:, :NG],
                    axis=mybir.AxisListType.X, op=Alu.add,
                )
                nc.vector.tensor_reduce(
                    out=sp[:, 1:2], in_=ssq[:, :NG],
                    axis=mybir.AxisListType.X, op=Alu.add,
                )
                nc.sync.dma_start(out=st_in[layer][:], in_=sp[:])
                nc.gpsimd.collective_compute(
                    "AllReduce", mybir.AluOpType.add,
                    replica_groups=[list(range(C))],
                    ins=[st_in[layer].opt()], outs=[st_out[layer].opt()],
                )
                pass

            # ---- output: affine + transpose + store ----
            compute_fold(2)
            out_big = cpool.tile(
                [128, n_pool_chunks * 3 * D], dt.float32, name="out_big"
            )
            with tc.tile_pool(name="poolt", bufs=2) as ptpool:
                for l in range(3):
                    # pooled z2 = relu(max(m2) + b2); then BN affine
                    pre = ptpool.tile([128, GPC], dt.float32, name="pre")
                    nc.scalar.activation(
                        pre[:], pt_all[l][:], Act.Relu, bias=b2_sb[l][:]
                    )
                    pta = ptpool.tile([128, GPC], dt.float32, name="pta")
                    nc.vector.tensor_scalar(
                        out=pta[:], in0=pre[:],
                        scalar1=s_all[:, l : l + 1],
                        scalar2=t_all[:, l : l + 1], op0=Alu.mult, op1=Alu.add,
                    )
                    for ch in range(n_pool_chunks):
                        rows = (
                            128 if ch < n_pool_chunks - 1 else last_chunk_rows
                        )
                        trp = trpool.tile(
                            [128, 128], dt.float32, name="trpo", tag="tr"
                        )
                        nc.tensor.transpose(
                            trp[:rows, :],
                            pta[:, ch * 128 : ch * 128 + rows], ident32[:],
                        )
                        nc.any.tensor_copy(
                            out=out_big[
                                :rows, ch * 3 * D + l * D : ch * 3 * D
                                + (l + 1) * D
                            ],
                            in_=trp[:rows, :],
                        )
            for ch in range(n_pool_chunks):
                rows = 128 if ch < n_pool_chunks - 1 else last_chunk_rows
                nc.sync.dma_start(
                    out=out_d[ch * 128 : ch * 128 + rows, :],
                    in_=out_big[:rows, ch * 3 * D : (ch + 1) * 3 * D],
                )

    nc.compile()
    for l in range(2):
        base = None
        for k in range(N_CHUNKS):
            mls = nc.lookup_mls(h_ch_handles[l][k])
            addr = mls.memorylocations[0].addr
            shq_k = mls.tensor_shape[0]
            if base is not None:
                assert addr == base, (
                    f"h chunks not contiguous at l={l} k={k}: {addr} != {base}"
                )
            base = addr + shq_k * DIM * 2
    return nc, input_names


def make_in_maps(hd: HostData, inputs: dict, input_names):
    iota = np.tile(np.arange(128, dtype=np.float32), (128, 1)).astype(np_bf16)
    gb = np.zeros((DIM, 6), dtype=np.float32)
    for l in range(3):
        gb[:, 2 * l] = inputs["gamma"][l]
        gb[:, 2 * l + 1] = inputs["beta"][l]
    shared = {
        "x_tbl": hd.x_tbl,
        "iota": np.ascontiguousarray(iota),
        "gb": gb,
    }
    for l in range(3):
        w = np.zeros((DIM, DIM), dtype=np.float32)
        wl = inputs[f"w1_{l}"]
        w[: wl.shape[0], :] = wl
        shared[f"w1_{l}"] = w
        shared[f"w2_{l}"] = np.ascontiguousarray(
            inputs[f"w2_{l}"].astype(np.float32)
        )
        shared[f"b1_{l}"] = inputs[f"b1_{l}"].astype(np.float32).reshape(-1, 1)
        shared[f"b2_{l}"] = inputs[f"b2_{l}"].astype(np.float32).reshape(-1, 1)
    in_maps = []
    for c in range(N_CORES):
        m = dict(shared)
        m["idx"] = hd.idx_sb[c]
        m["rel"] = hd.rel_sb[c]
        m["deg2"] = hd.deg2[c]
        m["xT"] = hd.xT[c]
        assert set(m.keys()) == set(input_names)
        in_maps.append(m)
    return in_maps


def _run_sharded_timed(nc, in_maps, n_cores, iters=10, warmup=2):
    """Execute the compiled Bass module via PJRT with device-resident inputs,
    timing `iters` back-to-back dispatches (excludes input upload/compile)."""
    import time

    import jax
    from jax.sharding import Mesh, NamedSharding, PartitionSpec
    from jax.experimental.shard_map import shard_map

    import concourse.mybir as mybir
    from concourse import bass2jax

    bass2jax.install_neuronx_cc_hook()
    partition_name = (
        nc.partition_id_tensor.name if nc.partition_id_tensor else None
    )
    in_names, out_names, out_avals, zero_outs = [], [], [], []
    for alloc in nc.m.functions[0].allocations:
        if not isinstance(alloc, mybir.MemoryLocationSet):
            continue
        name = alloc.memorylocations[0].name
        if alloc.kind == "ExternalInput":
            if name != partition_name:
                in_names.append(name)
        elif alloc.kind == "ExternalOutput":
            out_names.append(name)
            shape = tuple(alloc.tensor_shape)
            dtp = mybir.dt.np(alloc.dtype)
            out_avals.append(jax.core.ShapedArray(shape, dtp))
            zero_outs.append(np.zeros(shape, dtp))
    n_params, n_outs = len(in_names), len(out_avals)
    in_names.extend(out_names)
    if partition_name is not None:
        in_names.append(partition_name)
    donate = tuple(range(n_params, n_params + n_outs))

    def _body(*args):
        operands = list(args)
        if partition_name is not None:
            operands.append(bass2jax.partition_id_tensor())
        outs = bass2jax._bass_exec_p.bind(
            *operands,
            out_avals=tuple(out_avals),
            in_names=tuple(in_names),
            out_names=tuple(out_names),
            lowering_input_output_aliases=(),
            sim_require_finite=True,
            sim_require_nnan=True,
            nc=nc,
        )
        return tuple(outs)

    devices = jax.devices()[:n_cores]
    mesh = Mesh(np.asarray(devices), ("core",))
    pspec = PartitionSpec("core")
    in_specs = (pspec,) * (n_params + n_outs)
    sharded = jax.jit(
        shard_map(
            _body, mesh=mesh, in_specs=in_specs,
            out_specs=(pspec,) * len(out_names), check_rep=False,
        ),
        donate_argnums=donate, keep_unused=True,
    )
    shd = NamedSharding(mesh, pspec)
    per_core = [
        [np.asarray(m[name]) for name in in_names[:n_params]] for m in in_maps
    ]
    dev_in = [
        jax.device_put(
            np.concatenate([per_core[c][i] for c in range(n_cores)], axis=0),
            shd,
        )
        for i in range(n_params)
    ]
    n_calls = warmup + (iters if iters else 0)
    zsets = [
        [
            jax.device_put(
                np.zeros((n_cores * z.shape[0], *z.shape[1:]), z.dtype), shd
            )
            for z in zero_outs
        ]
        for _ in range(max(n_calls, 1))
    ]
    outs = None
    for i in range(warmup):
        outs = sharded(*dev_in, *zsets[i])
        jax.block_until_ready(outs)
    dt = None
    if iters:
        t0 = time.perf_counter()
        ress = [sharded(*dev_in, *zsets[warmup + i]) for i in range(iters)]
        jax.block_until_ready(ress)
        dt = (time.perf_counter() - t0) / iters
        outs = ress[-1]
    if outs is None:
        outs = sharded(*dev_in, *zsets[0])
    results = [
        {
            name: np.asarray(outs[i]).reshape(n_cores, *out_avals[i].shape)[c]
            for i, name in enumerate(out_names)
        }
        for c in range(n_cores)
    ]
    return results, dt


def run(inputs: dict, timed: bool = False):
    x = np.asarray(inputs["x"])
    ei = np.asarray(inputs["edge_index"])
    batch = np.asarray(inputs["batch"])
    hd = prep_host(x, ei, batch)
    nc, input_names = build_program(hd)
    in_maps = make_in_maps(hd, inputs, input_names)
    results, dt = _run_sharded_timed(
        nc, in_maps, N_CORES,
        iters=(10 if timed else 0), warmup=(2 if timed else 1),
    )
    outs = [results[c]["pooled"] for c in range(N_CORES)]
    full = np.concatenate(outs, axis=0).astype(np.float32)
    return full, dt


def kernel(**inputs) -> np.ndarray:
    out, _ = run(inputs, timed=False)
    return out



# revision 14
# speedup vs baseline: 2.3456x; 2.3456x over previous
"""GIN (3-layer) message-passing kernel for Trainium2, 8 NeuronCores.

v2 — batched-gather rewrite of the graph-partition data-parallel design.

  - Graphs assigned to cores by id (750 graphs x 50 nodes per core); nodes
    renumbered into a chunk-interleaved shared h table (4 chunks per layer
    for overlapped AllGathers).  The GIN self term comes from an
    SBUF-resident feat-major copy (zkeep), BN is folded into the next
    layer's first matmul (scale + rank-1 degree correction), stats come
    free from activation accum_out and a 1KB AllReduce.
  - Aggregation (the v2 part): edges are sharded by destination core and
    grouped by (512-slot destination group, 32768-row source window).
    Each (cohort of 3 groups, window) produces one int16 dma_gather call
    (<=1024 indices) instead of per-128-edge indirect DMAs: the SWDGE
    offset walker only supports one offset column per indirect call
    (~1.3us/call measured), while dma_gather moves ~5-6ns/row.  Gathered
    k-tiles (128 edges) are reduced into per-group PSUM banks by one-hot
    matmuls ([128e,128f]^T x [128e,512slots], is_equal-built one-hots).
  - MLP runs in transposed space per 512-col group (fp32r), pooling
    on-the-fly from raw m2 (max commutes with the final monotone affine),
    transposes feed the next layer's node-major h table.
Host assembles the 8 per-core [750, 384] outputs into the full [6000, 384].
"""

import sys

sys.path.insert(0, "/opt/trn_rl_repo")

import math
from dataclasses import dataclass, field

import numpy as np

try:
    from ml_dtypes import bfloat16 as np_bf16
except ImportError:  # pragma: no cover
    import jax.numpy as _jnp

    np_bf16 = _jnp.bfloat16

N_GRAPHS = 6000
N_CORES = 8
IN_DIM = 77
DIM = 128
EPS = 1e-5
GRP = 512  # slots per PSUM aggregation group
COH = 3  # groups per cohort (agg PSUM banks in flight)
WIN = 32768  # table rows per int16 gather window
CAP_TILES = 6  # max k-tiles (128 idxs each) per dma_gather call
N_CHUNKS = 1  # single chunk: Shared DRAM allows one writer inst


@dataclass
class HostData:
    gs: int
    gpc: int
    slots: int
    nb: int
    shp: int
    kt_total: int
    nw: int
    cohorts: list  # per cohort: {'groups': [g..], 'calls': [(w, icol0, [(g, kt0, ntiles)..])..]}
    tiles_per_group: np.ndarray  # [NG]
    icols: int
    idx16: list  # per core [128, icols] int16
    relc: list  # per core [128, KT] f32
    degt: list  # per core [128, GRP] bf16
    x_tbl: np.ndarray  # [TBL, 128] bf16
    cb: np.ndarray  # chunk boundaries in blocks
    xT: list  # per core [128, SHP] bf16

    @property
    def tbl(self):
        return N_CORES * self.shp

    @property
    def ng(self):
        return (self.shp + GRP - 1) // GRP


def prep_host(x: np.ndarray, edge_index: np.ndarray, batch: np.ndarray) -> HostData:
    C = N_CORES
    N = x.shape[0]
    batch = batch.astype(np.int64)
    sizes = np.bincount(batch, minlength=N_GRAPHS)
    assert sizes.min() >= 1
    starts = np.concatenate([[0], np.cumsum(sizes)[:-1]])
    GS = int(sizes.max())
    GPC = N_GRAPHS // C
    SLOTS = GPC * GS
    NB = (SLOTS + 127) // 128
    SHP = NB * 128
    TBL = C * SHP
    NG = (SHP + GRP - 1) // GRP
    NW = (TBL + WIN - 1) // WIN

    # chunked table layout: chunk k holds blocks [cb[k], cb[k+1]) of all cores
    nb_ch = [NB // N_CHUNKS + (1 if i < NB % N_CHUNKS else 0) for i in range(N_CHUNKS)]
    cb = np.concatenate([[0], np.cumsum(nb_ch)]).astype(np.int64)
    shq = [int(n) * 128 for n in nb_ch]
    chunk_base = np.concatenate([[0], np.cumsum([C * s for s in shq])]).astype(np.int64)

    def slot_to_row(core, slot):
        blk = slot >> 7
        k = np.searchsorted(cb[1:], blk, side="right")
        return chunk_base[k] + core * np.asarray(shq)[k] + (slot - cb[k] * 128)

    g_of = batch
    pos = np.arange(N, dtype=np.int64) - starts[g_of]
    core_of = g_of // GPC
    slot_loc = (g_of - core_of * GPC) * GS + pos
    row_of = slot_to_row(core_of, slot_loc).astype(np.int64)

    src = edge_index[0].astype(np.int64)
    dst = edge_index[1].astype(np.int64)

    e_core = [core_of[dst]]
    e_dslot = [slot_loc[dst]]
    e_srow = [row_of[src]]

    # duplicate slots: graph g's pad slots [size_g, GS) copy n0 = starts[g]
    n0_edges = np.where(dst == starts[g_of[dst]])[0]
    n0_g = g_of[dst[n0_edges]]
    max_pad = GS - int(sizes.min())
    for j in range(max_pad):
        gsel_mask = sizes + j < GS
        em = gsel_mask[n0_g]
        gg = n0_g[em]
        pc = gg // GPC
        ps = (gg - pc * GPC) * GS + sizes[gg] + j
        e_core.append(pc)
        e_dslot.append(ps)
        e_srow.append(row_of[src[n0_edges[em]]])

    e_core = np.concatenate(e_core)
    e_dslot = np.concatenate(e_dslot)
    e_srow = np.concatenate(e_srow)

    # per-core, per (group, window) edge cells
    per_core = []
    cnt = np.zeros((C, NG, NW), dtype=np.int64)
    for c in range(C):
        m = e_core == c
        ds, sr = e_dslot[m], e_srow[m]
        g_e = ds // GRP
        w_e = sr >> 15
        # sort by (g, w, srow) for cell grouping + ascending-address locality
        order = np.lexsort((sr, w_e, g_e))
        ds, sr, g_e, w_e = ds[order], sr[order], g_e[order], w_e[order]
        np.add.at(cnt[c], (g_e, w_e), 1)
        per_core.append((ds, sr, g_e, w_e))

    ntile_gw = (cnt.max(axis=0) + 127) // 128  # [NG, NW], shared structure
    tiles_per_group = ntile_gw.sum(axis=1).astype(np.int64)
    assert (tiles_per_group > 0).all()

    # build shared call structure: cohort -> window -> calls of <= CAP_TILES
    cohorts = []
    kt = 0
    icol = 0
    for c0g in range(0, NG, COH):
        groups = list(range(c0g, min(c0g + COH, NG)))
        coh = {"groups": groups, "calls": []}
        for w in range(NW):
            # flat tile list for this (cohort, w)
            tlist = []  # (g,) one entry per k-tile
            for g in groups:
                tlist += [g] * int(ntile_gw[g, w])
            i = 0
            while i < len(tlist):
                chunk = tlist[i : i + CAP_TILES]
                cells = []
                j = 0
                while j < len(chunk):
                    g = chunk[j]
                    n = 1
                    while j + n < len(chunk) and chunk[j + n] == g:
                        n += 1
                    cells.append((g, kt, n))
                    kt += n
                    j += n
                coh["calls"].append((w, icol, cells))
                icol += len(chunk) * 8  # 128 idxs per tile / 16 rows
                i += len(chunk)
        cohorts.append(coh)
    KT = kt
    ICOLS = icol

    # per-core idx16 / relc
    idx16_l, relc_l, degt_l = [], [], []
    for c in range(C):
        ds, sr, g_e, w_e = per_core[c]
        # cell start offsets per (g, w)
        cell_cnt = cnt[c]
        cell_start = np.zeros((NG, NW), dtype=np.int64)
        flat = (g_e * NW + w_e).astype(np.int64)
        # edges sorted by (g, w) already; starts via cumsum over cells
        cc = np.zeros(NG * NW, dtype=np.int64)
        np.add.at(cc, flat, 1)
        cs = np.concatenate([[0], np.cumsum(cc)[:-1]]).reshape(NG, NW)
        cell_start = cs

        idx_lin = np.zeros(KT * 128, dtype=np.int16)
        rel_arr = np.full((128, KT), -1.0, dtype=np.float32)
        consumed = np.zeros((NG, NW), dtype=np.int64)
        for coh in cohorts:
            for (w, icol0, cells) in coh["calls"]:
                for (g, kt0, ntiles) in cells:
                    base = cell_start[g, w]
                    tot = cell_cnt[g, w]
                    for j in range(ntiles):
                        t = kt0 + j
                        lo = consumed[g, w]
                        hi = min(lo + 128, tot)
                        nreal = hi - lo
                        consumed[g, w] = hi
                        sl = slice(base + lo, base + hi)
                        locs = (sr[sl] & (WIN - 1)).astype(np.int16)
                        rels = (ds[sl] % GRP).astype(np.float32)
                        seg = np.zeros(128, dtype=np.int16)
                        seg[:nreal] = locs
                        if nreal > 0 and nreal < 128:
                            seg[nreal:] = locs[-1] if nreal else 0
                        idx_lin[t * 128 : t * 128 + 128] = seg
                        rel_arr[:nreal, t] = rels
        # wrap idx per call block: idx i of call -> (i%16, i//16), replicate x8
        idx16 = np.zeros((128, ICOLS), dtype=np.int16)
        for coh in cohorts:
            for (w, icol0, cells) in coh["calls"]:
                nt = sum(n for (_, _, n) in cells)
                n = nt * 128
                kt0 = cells[0][1]
                blockv = idx_lin[kt0 * 128 : kt0 * 128 + n]
                wrapped = blockv.reshape(n // 16, 16).T  # [16, n/16]
                idx16[:, icol0 : icol0 + n // 16] = np.tile(wrapped, (8, 1))
        idx16_l.append(idx16)
        relc_l.append(np.ascontiguousarray(rel_arr))

        # per-slot degree for the rank-1 BN fold: rows at partitions 0/32/64
        deg_p = np.bincount(ds, minlength=NG * GRP).astype(np.float32)
        deg_p[:SLOTS] += 1.0
        deg_p[SLOTS:] = 0.0
        ncol3 = (NG + 2) // 3
        dg = np.zeros((128, ncol3 * GRP), dtype=np.float32)
        for g in range(NG):
            dg[(g % 3) * 32, (g // 3) * GRP : (g // 3 + 1) * GRP] = deg_p[
                g * GRP : (g + 1) * GRP
            ]
        degt_l.append(dg.astype(np_bf16))

    x_tbl = np.zeros((TBL, 128), dtype=np_bf16)
    x_tbl[row_of, :IN_DIM] = x.astype(np_bf16)

    # per-core feat-major x (incl dup slots), for the SBUF self-add
    xT = []
    xs = x.astype(np.float32)
    for c in range(C):
        xt = np.zeros((128, SHP), dtype=np.float32)
        m = core_of == c
        xt[:IN_DIM, slot_loc[m]] = xs[m].T
        gsel = np.arange(N_GRAPHS)[(np.arange(N_GRAPHS) // GPC) == c]
        for g in gsel:
            sz = sizes[g]
            if sz < GS:
                base = (g - c * GPC) * GS
                xt[:IN_DIM, base + sz : base + GS] = xs[starts[g]][:, None]
        xT.append(xt.astype(np_bf16))

    return HostData(
        GS, GPC, SLOTS, NB, SHP, KT, NW, cohorts, tiles_per_group, ICOLS,
        idx16_l, relc_l, degt_l, x_tbl, cb, xT,
    )


def build_program(hd: HostData):
    """Returns (nc, input_names)."""
    import concourse.bass as bass
    import concourse.mybir as mybir
    import concourse.tile as tile
    from concourse import bacc
    from concourse.masks import make_identity
    from concourse.tile_rust import add_dep_helper

    dt = mybir.dt
    Alu = mybir.AluOpType
    Act = mybir.ActivationFunctionType

    C, D = N_CORES, DIM
    NB, SHP, TBL, NG, KT = hd.nb, hd.shp, hd.tbl, hd.ng, hd.kt_total
    GS, GPC, SLOTS, NW = hd.gs, hd.gpc, hd.slots, hd.nw
    inv_n = 1.0 / (C * SLOTS)

    nc = bacc.Bacc(
        "TRN2", target_bir_lowering=False, debug=False, num_devices=C
    )

    def din(name, shape, dtp=dt.float32):
        return nc.dram_tensor(name, list(shape), dtp, kind="ExternalInput").ap()

    x_tbl_d = din("x_tbl", (TBL, D), dt.bfloat16)
    xT_d = din("xT", (128, SHP), dt.bfloat16)
    idx_d = din("idx", (128, hd.icols), dt.int16)
    rel_d = din("rel", (128, KT))
    ncol3 = (NG + 2) // 3
    degt_d = din("degt", (128, ncol3 * GRP), dt.bfloat16)
    iota_d = din("iota", (128, GRP))
    w1_d = [din(f"w1_{l}", (D, D)) for l in range(3)]
    w2_d = [din(f"w2_{l}", (D, D)) for l in range(3)]
    b1_d = [din(f"b1_{l}", (D, 1)) for l in range(3)]
    b2_d = [din(f"b2_{l}", (D, 1)) for l in range(3)]
    gb_d = din("gb", (D, 6))
    out_d = nc.dram_tensor(
        "pooled", [GPC, 3 * D], dt.float32, kind="ExternalOutput"
    ).ap()

    input_names = (
        ["x_tbl", "xT", "idx", "rel", "degt", "iota"]
        + [f"w1_{l}" for l in range(3)]
        + [f"w2_{l}" for l in range(3)]
        + [f"b1_{l}" for l in range(3)]
        + [f"b2_{l}" for l in range(3)]
        + ["gb"]
    )

    n_pool_chunks = (GPC + 127) // 128
    last_chunk_rows = GPC - (n_pool_chunks - 1) * 128

    with tile.TileContext(nc) as tc:
        with (
            tc.tile_pool(name="const", bufs=1) as cpool,
            tc.tile_pool(name="ebuf", bufs=10) as epool,
            tc.tile_pool(name="spool", bufs=8) as spool,
            tc.tile_pool(name="zin", bufs=2) as zinpool,
            tc.tile_pool(name="zmid", bufs=2) as zmidpool,
            tc.tile_pool(name="rm", bufs=3) as rmpool,
            tc.tile_pool(name="stat", bufs=1) as statpool,
            tc.tile_pool(name="agg_ps", bufs=COH, space="PSUM") as aggpool,
            tc.tile_pool(name="m1_ps", bufs=2, space="PSUM") as m1pool,
            tc.tile_pool(name="m2_ps", bufs=2, space="PSUM") as m2pool,
            tc.tile_pool(name="tr_ps", bufs=1, space="PSUM") as trpool,
            tc.tile_pool(name="dram", bufs=1, space="DRAM") as dpool,
        ):
            # ---- DRAM intermediates ----
            cb = [int(v) for v in hd.cb]
            NCH = N_CHUNKS
            shq = [(cb[k + 1] - cb[k]) * 128 for k in range(NCH)]
            chunk_base = [0]
            for k in range(NCH):
                chunk_base.append(chunk_base[-1] + C * shq[k])
            h_tbl = [
                dpool.tile([TBL, D], dt.bfloat16, name=f"h_{l}", addr_space="Shared")
                for l in range(2)
            ]
            z_ch = [
                dpool.tile([shq[k], D], dt.bfloat16, name=f"z_ch{k}")
                for k in range(NCH)
            ]
            st_in = [
                dpool.tile([D, 2], dt.float32, name=f"st_in{l}") for l in range(3)
            ]
            st_out = [
                dpool.tile([D, 2], dt.float32, name=f"st_out{l}")
                for l in range(3)
            ]

            # ---- constants to SBUF ----
            def load(shape, src_ap, dtp=dt.float32, name=None):
                t = cpool.tile(list(shape), dtp, name=name)
                nc.sync.dma_start(out=t[:], in_=src_ap)
                return t

            idx_sb = load((128, hd.icols), idx_d[:], dt.int16, name="idx_sb")
            rel_sb = load((128, KT), rel_d[:], name="rel_sb")
            degt_sb = load(
                (128, ncol3 * GRP), degt_d[:], dt.bfloat16, name="degt_sb"
            )
            iota_sb = load((128, GRP), iota_d[:], name="iota_sb")
            w1_sb = [load((D, D), w1_d[l][:], name=f"w1sb{l}") for l in range(3)]
            w2_sb = [load((D, D), w2_d[l][:], name=f"w2sb{l}") for l in range(3)]
            b1_sb = [load((D, 1), b1_d[l][:], name=f"b1sb{l}") for l in range(3)]
            b2_sb = [load((D, 1), b2_d[l][:], name=f"b2sb{l}") for l in range(3)]
            gb_sb = load((D, 6), gb_d[:], name="gb_sb")
            w1r0 = cpool.tile([D, D], dt.float32, name="w1r0")
            nc.any.tensor_copy(out=w1r0[:], in_=w1_sb[0][:])
            w2r = []
            for l in range(3):
                t = cpool.tile([D, D], dt.float32, name=f"w2r{l}")
                nc.any.tensor_copy(out=t[:], in_=w2_sb[l][:])
                w2r.append(t)
            ident = cpool.tile([128, 128], dt.bfloat16, name="ident")
            make_identity(nc, ident[:])
            ident32 = cpool.tile([128, 128], dt.float32, name="ident32")
            make_identity(nc, ident32[:])

            s_all = cpool.tile([D, 3], dt.float32, name="s_all")
            t_all = cpool.tile([D, 3], dt.float32, name="t_all")
            w1s_sb = [
                cpool.tile([D, D], dt.float32, name=f"w1s{l}") for l in (1, 2)
            ]
            u_sb = [cpool.tile([1, D], dt.float32, name=f"u{l}") for l in (1, 2)]
            ub_sb = [
                cpool.tile([D, D], dt.bfloat16, name=f"ub{l}") for l in (1, 2)
            ]
            ones_row = cpool.tile([1, D], dt.float32, name="ones_row")
            nc.gpsimd.memset(ones_row[:], 1.0)
            ssum = cpool.tile([128, NG], dt.float32, name="ssum")
            ssq = cpool.tile([128, NG], dt.float32, name="ssq")
            sq_scr = cpool.tile([128, GRP], dt.float32, name="sq_scr")
            stat_scr = cpool.tile([128, 8], dt.float32, name="stat_scr")
            pt_all = [
                cpool.tile([128, GPC], dt.float32, name=f"pt{l}")
                for l in range(3)
            ]
            zkeep = cpool.tile([128, SHP], dt.bfloat16, name="zkeep")
            nc.sync.dma_start(out=zkeep[:], in_=xT_d[:])

            def compute_fold(l):
                st = statpool.tile([D, 2], dt.float32, name="st_ld")
                nc.sync.dma_start(out=st[:], in_=st_out[l][:])
                mu = stat_scr[:, 0:1]
                msq = stat_scr[:, 1:2]
                var = stat_scr[:, 2:3]
                rstd = stat_scr[:, 3:4]
                smu = stat_scr[:, 4:5]
                nc.vector.tensor_scalar_mul(mu, st[:, 0:1], inv_n)
                nc.vector.tensor_scalar_mul(msq, st[:, 1:2], inv_n)
                nc.vector.tensor_tensor(out=var, in0=mu, in1=mu, op=Alu.mult)
                nc.vector.tensor_tensor(
                    out=var, in0=msq, in1=var, op=Alu.subtract
                )
                veps = stat_scr[:, 6:7]
                nc.vector.tensor_scalar_add(veps, var, EPS)
                std = stat_scr[:, 5:6]
                nc.scalar.activation(std, veps, Act.Sqrt)
                nc.vector.reciprocal(rstd, std)
                scol = s_all[:, l : l + 1]
                tcol = t_all[:, l : l + 1]
                nc.vector.tensor_tensor(
                    out=scol, in0=gb_sb[:, 2 * l : 2 * l + 1], in1=rstd,
                    op=Alu.mult,
                )
                nc.vector.tensor_tensor(out=smu, in0=scol, in1=mu, op=Alu.mult)
                nc.vector.tensor_tensor(
                    out=tcol, in0=gb_sb[:, 2 * l + 1 : 2 * l + 2], in1=smu,
                    op=Alu.subtract,
                )
                if l < 2:
                    ln = l + 1
                    nc.vector.tensor_scalar(
                        out=w1s_sb[ln - 1][:], in0=w1_sb[ln][:], scalar1=scol,
                        scalar2=None, op0=Alu.mult,
                    )
                    ups = trpool.tile([1, D], dt.float32, name="ups", tag="tr")
                    nc.tensor.matmul(
                        ups[:], lhsT=tcol, rhs=w1_sb[ln][:], start=True,
                        stop=True,
                    )
                    nc.any.tensor_copy(out=u_sb[ln - 1][:], in_=ups[:])
                    ubp = trpool.tile([D, D], dt.float32, name="ubp", tag="tr")
                    nc.tensor.matmul(
                        ubp[:], lhsT=ones_row[:], rhs=u_sb[ln - 1][:],
                        start=True, stop=True,
                    )
                    nc.any.tensor_copy(out=ub_sb[ln - 1][:], in_=ubp[:])

            def win_ap(tensor_ap, w):
                wl = min(WIN, TBL - w * WIN)
                return tensor_ap[w * WIN : w * WIN + wl, :]

            ag_insts = [[], []]
            for layer in range(3):
                if layer > 0:
                    compute_fold(layer - 1)
                lhs1 = w1r0 if layer == 0 else w1s_sb[layer - 1]
                pt = pt_all[layer]
                tbl_ap = x_tbl_d if layer == 0 else h_tbl[layer - 1][:]

                kt_done = np.zeros(NG, dtype=np.int64)
                first_gather = True
                for coh in hd.cohorts:
                    aggt = {}
                    for (w, icol0, cells) in coh["calls"]:
                        nt = sum(n for (_, _, n) in cells)
                        n = nt * 128
                        et = epool.tile(
                            [128, CAP_TILES * 128], dt.bfloat16, name="ebuf"
                        )
                        gi = nc.gpsimd.dma_gather(
                            et[:, :n].rearrange("p (t f) -> p t f", f=128),
                            win_ap(tbl_ap, w),
                            idx_sb[:, icol0 : icol0 + n // 16],
                            n,
                            n,
                            128,
                        )
                        if first_gather:
                            first_gather = False
                            if layer > 0:
                                for agi in ag_insts[layer - 1]:
                                    add_dep_helper(
                                        getattr(gi, "ins", gi),
                                        getattr(agi, "ins", agi),
                                        reason="gather waits h AllGather",
                                    )
                        col = 0
                        for (g, kt0, ntiles) in cells:
                            W = min(GRP, SHP - g * GRP)
                            if g not in aggt:
                                aggt[g] = aggpool.tile(
                                    [128, GRP], dt.float32, name="agg"
                                )
                            for j in range(ntiles):
                                ktg = kt0 + j
                                s_t = spool.tile(
                                    [128, GRP], dt.bfloat16, name="s_t"
                                )
                                nc.vector.tensor_scalar(
                                    out=s_t[:, :W], in0=iota_sb[:, :W],
                                    scalar1=rel_sb[:, ktg : ktg + 1],
                                    scalar2=None, op0=Alu.is_equal,
                                )
                                nc.tensor.matmul(
                                    aggt[g][:, :W],
                                    lhsT=et[:, col : col + 128],
                                    rhs=s_t[:, :W],
                                    start=(kt_done[g] == 0),
                                    stop=(
                                        kt_done[g] + 1
                                        == hd.tiles_per_group[g]
                                    ),
                                )
                                kt_done[g] += 1
                                col += 128
                    # ---- MLP on the cohort's groups ----
                    for g in coh["groups"]:
                        c0 = g * GRP
                        W = min(GRP, SHP - c0)
                        zin = zinpool.tile([128, GRP], dt.float32, name="zin")
                        nc.vector.tensor_tensor(
                            out=zin[:, :W], in0=aggt[g][:, :W],
                            in1=zkeep[:, c0 : c0 + W], op=Alu.add,
                        )
                        m1 = m1pool.tile([128, GRP], dt.float32, name="m1")
                        nc.tensor.matmul(
                            m1[:, :W], lhsT=lhs1[:], rhs=zin[:, :W],
                            start=True, stop=(layer == 0),
                        )
                        if layer > 0:
                            dp = (g % 3) * 32
                            dc = (g // 3) * GRP
                            nc.tensor.matmul(
                                m1[:, :W],
                                lhsT=ub_sb[layer - 1][dp : dp + 1, :],
                                rhs=degt_sb[dp : dp + 1, dc : dc + W],
                                start=False, stop=True,
                            )
                        z1 = zmidpool.tile([128, GRP], dt.float32, name="z1")
                        nc.scalar.activation(
                            z1[:, :W], m1[:, :W], Act.Relu, bias=b1_sb[layer][:]
                        )
                        m2 = m2pool.tile([128, GRP], dt.float32, name="m2")
                        nc.tensor.matmul(
                            m2[:, :W], lhsT=w2r[layer][:], rhs=z1[:, :W],
                            start=True, stop=True,
                        )
                        z2 = zkeep[:, c0 : c0 + W]
                        wr = min(W, max(0, SLOTS - c0))
                        if wr > 0:
                            nc.scalar.activation(
                                z2[:, :wr], m2[:, :wr], Act.Relu,
                                bias=b2_sb[layer][:],
                                accum_out=ssum[:, g : g + 1],
                            )
                        if wr < W:
                            nc.scalar.activation(
                                z2[:, wr:W], m2[:, wr:W], Act.Relu,
                                bias=b2_sb[layer][:],
                            )
                        if wr > 0:
                            nc.scalar.activation(
                                sq_scr[:, :wr], z2[:, :wr], Act.Square,
                                accum_out=ssq[:, g : g + 1],
                            )
                        # ---- on-the-fly pooling (raw m2; relu+b2 at end) ----
                        pc1 = min(c0 + W, SLOTS)
                        if c0 < pc1:
                            gfirst = (c0 + GS - 1) // GS
                            a = gfirst * GS - c0
                            gend = pc1 // GS
                            nfull = gend - gfirst
                            if nfull > 0:
                                nc.vector.tensor_reduce(
                                    out=pt[:, gfirst:gend],
                                    in_=m2[:, a : a + nfull * GS].rearrange(
                                        "p (g s) -> p g s", s=GS
                                    ),
                                    axis=mybir.AxisListType.X, op=Alu.max,
                                )
                            if a > 0:
                                la = min(a, pc1 - c0)
                                tmpm = stat_scr[:, 7:8]
                                nc.vector.tensor_reduce(
                                    out=tmpm, in_=m2[:, 0:la],
                                    axis=mybir.AxisListType.X, op=Alu.max,
                                )
                                gl = gfirst - 1
                                nc.vector.tensor_tensor(
                                    out=pt[:, gl : gl + 1],
                                    in0=pt[:, gl : gl + 1], in1=tmpm,
                                    op=Alu.max,
                                )
                            r0 = a + max(0, gend - gfirst) * GS
                            if gend >= gfirst and c0 + r0 < pc1:
                                nc.vector.tensor_reduce(
                                    out=pt[:, gend : gend + 1],
                                    in_=m2[:, r0 : pc1 - c0],
                                    axis=mybir.AxisListType.X, op=Alu.max,
                                )
                        # ---- transpose to node-major for the h table ----
                        if layer < 2:
                            for i in range(W // 128):
                                trp = trpool.tile(
                                    [128, 128], dt.bfloat16, name="trp",
                                    tag="tr",
                                )
                                nc.tensor.transpose(
                                    trp[:], z2[:, i * 128 : (i + 1) * 128],
                                    ident[:],
                                )
                                rm = rmpool.tile(
                                    [128, 128], dt.bfloat16, name="rm"
                                )
                                nc.any.tensor_copy(out=rm[:], in_=trp[:])
                                b2i = c0 // 128 + i
                                kch = 0
                                while cb[kch + 1] <= b2i:
                                    kch += 1
                                lr0 = (b2i - cb[kch]) * 128
                                nc.sync.dma_start(
                                    out=z_ch[kch][lr0 : lr0 + 128, :],
                                    in_=rm[:],
                                )
                            # launch chunk AllGather as soon as blocks done
                            for kch in range(NCH):
                                if (cb[kch + 1] - 1) * 128 // GRP == g:
                                    agi = nc.gpsimd.collective_compute(
                                        "AllGather", mybir.AluOpType.bypass,
                                        replica_groups=[list(range(C))],
                                        ins=[z_ch[kch].opt()],
                                        outs=[
                                            h_tbl[layer][
                                                chunk_base[kch] : chunk_base[
                                                    kch
                                                ]
                                                + C * shq[kch],
                                                :,
                                            ].opt()
                                        ],
                                    )
                                    ag_insts[layer].append(agi)

                # ---- stats reduce + AllReduce ----
                sp = statpool.tile([D, 2], dt.float32, name="sp")
                nc.vector.tensor_reduce(
                    out=sp[:, 0:1], in_=ssum[:, :NG],
                    axis=mybir.AxisListType.X, op=Alu.add,
                )
                nc.vector.tensor_reduce(
                    out=sp[:, 1:2], in_=ssq[:, :NG],
                    axis=mybir.AxisListType.X, op=Alu.add,
                )
                nc.sync.dma_start(out=st_in[layer][:], in_=sp[:])
                nc.gpsimd.collective_compute(
                    "AllReduce", mybir.AluOpType.add,
                    replica_groups=[list(range(C))],
                    ins=[st_in[layer].opt()], outs=[st_out[layer].opt()],
                )

            # ---- output: affine + transpose + store ----
            compute_fold(2)
            out_big = cpool.tile(
                [128, n_pool_chunks * 3 * D], dt.float32, name="out_big"
            )
            with tc.tile_pool(name="poolt", bufs=2) as ptpool:
                for l in range(3):
                    pre = ptpool.tile([128, GPC], dt.float32, name="pre")
                    nc.scalar.activation(
                        pre[:], pt_all[l][:], Act.Relu, bias=b2_sb[l][:]
                    )
                    pta = ptpool.tile([128, GPC], dt.float32, name="pta")
                    nc.vector.tensor_scalar(
                        out=pta[:], in0=pre[:],
                        scalar1=s_all[:, l : l + 1],
                        scalar2=t_all[:, l : l + 1], op0=Alu.mult, op1=Alu.add,
                    )
                    for ch in range(n_pool_chunks):
                        rows = (
                            128 if ch < n_pool_chunks - 1 else last_chunk_rows
                        )
                        trp = trpool.tile(
                            [128, 128], dt.float32, name="trpo", tag="tr"
                        )
                        nc.tensor.transpose(
                            trp[:rows, :],
                            pta[:, ch * 128 : ch * 128 + rows], ident32[:],
                        )
                        nc.any.tensor_copy(
                            out=out_big[
                                :rows, ch * 3 * D + l * D : ch * 3 * D
                                + (l + 1) * D
                            ],
                            in_=trp[:rows, :],
                        )
            for ch in range(n_pool_chunks):
                rows = 128 if ch < n_pool_chunks - 1 else last_chunk_rows
                nc.sync.dma_start(
                    out=out_d[ch * 128 : ch * 128 + rows, :],
                    in_=out_big[:rows, ch * 3 * D : (ch + 1) * 3 * D],
                )

    nc.compile()
    return nc, input_names


def make_in_maps(hd: HostData, inputs: dict, input_names):
    iota = np.tile(np.arange(GRP, dtype=np.float32), (128, 1))
    gb = np.zeros((DIM, 6), dtype=np.float32)
    for l in range(3):
        gb[:, 2 * l] = inputs["gamma"][l]
        gb[:, 2 * l + 1] = inputs["beta"][l]
    shared = {
        "x_tbl": hd.x_tbl,
        "iota": np.ascontiguousarray(iota),
        "gb": gb,
    }
    for l in range(3):
        w = np.zeros((DIM, DIM), dtype=np.float32)
        wl = inputs[f"w1_{l}"]
        w[: wl.shape[0], :] = wl
        shared[f"w1_{l}"] = w
        shared[f"w2_{l}"] = np.ascontiguousarray(
            inputs[f"w2_{l}"].astype(np.float32)
        )
        shared[f"b1_{l}"] = inputs[f"b1_{l}"].astype(np.float32).reshape(-1, 1)
        shared[f"b2_{l}"] = inputs[f"b2_{l}"].astype(np.float32).reshape(-1, 1)
    in_maps = []
    for c in range(N_CORES):
        m = dict(shared)
        m["idx"] = hd.idx16[c]
        m["rel"] = hd.relc[c]
        m["degt"] = hd.degt[c]
        m["xT"] = hd.xT[c]
        assert set(m.keys()) == set(input_names)
        in_maps.append(m)
    return in_maps


def _run_sharded_timed(nc, in_maps, n_cores, iters=10, warmup=2):
    """Execute the compiled Bass module via PJRT with device-resident inputs,
    timing `iters` back-to-back dispatches (excludes input upload/compile)."""
    import time

    import jax
    from jax.sharding import Mesh, NamedSharding, PartitionSpec
    from jax.experimental.shard_map import shard_map

    import concourse.mybir as mybir
    from concourse import bass2jax

    bass2jax.install_neuronx_cc_hook()
    partition_name = (
        nc.partition_id_tensor.name if nc.partition_id_tensor else None
    )
    in_names, out_names, out_avals, zero_outs = [], [], [], []
    for alloc in nc.m.functions[0].allocations:
        if not isinstance(alloc, mybir.MemoryLocationSet):
            continue
        name = alloc.memorylocations[0].name
        if alloc.kind == "ExternalInput":
            if name != partition_name:
                in_names.append(name)
        elif alloc.kind == "ExternalOutput":
            out_names.append(name)
            shape = tuple(alloc.tensor_shape)
            dtp = mybir.dt.np(alloc.dtype)
            out_avals.append(jax.core.ShapedArray(shape, dtp))
            zero_outs.append(np.zeros(shape, dtp))
    n_params, n_outs = len(in_names), len(out_avals)
    in_names.extend(out_names)
    if partition_name is not None:
        in_names.append(partition_name)
    donate = tuple(range(n_params, n_params + n_outs))

    def _body(*args):
        operands = list(args)
        if partition_name is not None:
            operands.append(bass2jax.partition_id_tensor())
        outs = bass2jax._bass_exec_p.bind(
            *operands,
            out_avals=tuple(out_avals),
            in_names=tuple(in_names),
            out_names=tuple(out_names),
            lowering_input_output_aliases=(),
            sim_require_finite=True,
            sim_require_nnan=True,
            nc=nc,
        )
        return tuple(outs)

    devices = jax.devices()[:n_cores]
    mesh = Mesh(np.asarray(devices), ("core",))
    pspec = PartitionSpec("core")
    in_specs = (pspec,) * (n_params + n_outs)
    sharded = jax.jit(
        shard_map(
            _body, mesh=mesh, in_specs=in_specs,
            out_specs=(pspec,) * len(out_names), check_rep=False,
        ),
        donate_argnums=donate, keep_unused=True,
    )
    shd = NamedSharding(mesh, pspec)
    per_core = [
        [np.asarray(m[name]) for name in in_names[:n_params]] for m in in_maps
    ]
    dev_in = [
        jax.device_put(
            np.concatenate([per_core[c][i] for c in range(n_cores)], axis=0),
            shd,
        )
        for i in range(n_params)
    ]
    n_calls = warmup + (iters if iters else 0)
    zsets = [
        [
            jax.device_put(
                np.zeros((n_cores * z.shape[0], *z.shape[1:]), z.dtype), shd
            )
            for z in zero_outs
        ]
        for _ in range(max(n_calls, 1))
    ]
    jax.block_until_ready(zsets)
    jax.block_until_ready(dev_in)
    outs = None
    for i in range(warmup):
        outs = sharded(*dev_in, *zsets[i])
        jax.block_until_ready(outs)
    dt = None
    if iters:
        t0 = time.perf_counter()
        ress = [sharded(*dev_in, *zsets[warmup + i]) for i in range(iters)]
        jax.block_until_ready(ress)
        dt = (time.perf_counter() - t0) / iters
        outs = ress[-1]
    if outs is None:
        outs = sharded(*dev_in, *zsets[0])
    results = [
        {
            name: np.asarray(outs[i]).reshape(n_cores, *out_avals[i].shape)[c]
            for i, name in enumerate(out_names)
        }
        for c in range(n_cores)
    ]
    return results, dt


def run(inputs: dict, timed: bool = False):
    x = np.asarray(inputs["x"])
    ei = np.asarray(inputs["edge_index"])
    batch = np.asarray(inputs["batch"])
    hd = prep_host(x, ei, batch)
    nc, input_names = build_program(hd)
    in_maps = make_in_maps(hd, inputs, input_names)
    results, dt = _run_sharded_timed(
        nc, in_maps, N_CORES,
        iters=(100 if timed else 0), warmup=(4 if timed else 1),
    )
    outs = [results[c]["pooled"] for c in range(N_CORES)]
    full = np.concatenate(outs, axis=0).astype(np.float32)
    return full, dt


def kernel(**inputs) -> np.ndarray:
    out, _ = run(inputs, timed=False)
    return out


# revision 15
# speedup vs baseline: 2.5483x; 1.0864x over previous
"""GIN (3-layer) message-passing kernel for Trainium2, 8 NeuronCores.

v2 — batched-gather rewrite of the graph-partition data-parallel design.

  - Graphs assigned to cores by id (750 graphs x 50 nodes per core); nodes
    renumbered into a chunk-interleaved shared h table (4 chunks per layer
    for overlapped AllGathers).  The GIN self term comes from an
    SBUF-resident feat-major copy (zkeep), BN is folded into the next
    layer's first matmul (scale + rank-1 degree correction), stats come
    free from activation accum_out and a 1KB AllReduce.
  - Aggregation (the v2 part): edges are sharded by destination core and
    grouped by (512-slot destination group, 32768-row source window).
    Each (cohort of 3 groups, window) produces one int16 dma_gather call
    (<=1024 indices) instead of per-128-edge indirect DMAs: the SWDGE
    offset walker only supports one offset column per indirect call
    (~1.3us/call measured), while dma_gather moves ~5-6ns/row.  Gathered
    k-tiles (128 edges) are reduced into per-group PSUM banks by one-hot
    matmuls ([128e,128f]^T x [128e,512slots], is_equal-built one-hots).
  - MLP runs in transposed space per 512-col group (fp32r), pooling
    on-the-fly from raw m2 (max commutes with the final monotone affine),
    transposes feed the next layer's node-major h table.
Host assembles the 8 per-core [750, 384] outputs into the full [6000, 384].
"""

import sys

sys.path.insert(0, "/opt/trn_rl_repo")

import math
from dataclasses import dataclass, field

import numpy as np

try:
    from ml_dtypes import bfloat16 as np_bf16
except ImportError:  # pragma: no cover
    import jax.numpy as _jnp

    np_bf16 = _jnp.bfloat16

N_GRAPHS = 6000
N_CORES = 8
IN_DIM = 77
DIM = 128
EPS = 1e-5
GRP = 512  # slots per PSUM aggregation group
COH = 3  # groups per cohort (agg PSUM banks in flight)
WIN = 32768  # table rows per int16 gather window
CAP_TILES = 6  # max k-tiles (128 idxs each) per dma_gather call
N_CHUNKS = 1  # single chunk: Shared DRAM allows one writer inst


@dataclass
class HostData:
    gs: int
    gpc: int
    slots: int
    nb: int
    shp: int
    kt_total: int
    nw: int
    cohorts: list  # per cohort: {'groups': [g..], 'calls': [(w, icol0, [(g, kt0, ntiles)..])..]}
    tiles_per_group: np.ndarray  # [NG]
    icols: int
    idx16: list  # per core [128, icols] int16
    relc: list  # per core [128, KT] f32
    degt: list  # per core [128, GRP] bf16
    x_tbl: np.ndarray  # [TBL, 128] bf16
    cb: np.ndarray  # chunk boundaries in blocks
    xT: list  # per core [128, SHP] bf16

    @property
    def tbl(self):
        return N_CORES * self.shp

    @property
    def ng(self):
        return (self.shp + GRP - 1) // GRP


def prep_host(x: np.ndarray, edge_index: np.ndarray, batch: np.ndarray) -> HostData:
    C = N_CORES
    N = x.shape[0]
    batch = batch.astype(np.int64)
    sizes = np.bincount(batch, minlength=N_GRAPHS)
    assert sizes.min() >= 1
    starts = np.concatenate([[0], np.cumsum(sizes)[:-1]])
    GS = int(sizes.max())
    GPC = N_GRAPHS // C
    SLOTS = GPC * GS
    NB = (SLOTS + 127) // 128
    SHP = NB * 128
    TBL = C * SHP
    NG = (SHP + GRP - 1) // GRP
    NW = (TBL + WIN - 1) // WIN

    # chunked table layout: chunk k holds blocks [cb[k], cb[k+1]) of all cores
    nb_ch = [NB // N_CHUNKS + (1 if i < NB % N_CHUNKS else 0) for i in range(N_CHUNKS)]
    cb = np.concatenate([[0], np.cumsum(nb_ch)]).astype(np.int64)
    shq = [int(n) * 128 for n in nb_ch]
    chunk_base = np.concatenate([[0], np.cumsum([C * s for s in shq])]).astype(np.int64)

    def slot_to_row(core, slot):
        blk = slot >> 7
        k = np.searchsorted(cb[1:], blk, side="right")
        return chunk_base[k] + core * np.asarray(shq)[k] + (slot - cb[k] * 128)

    g_of = batch
    pos = np.arange(N, dtype=np.int64) - starts[g_of]
    core_of = g_of // GPC
    slot_loc = (g_of - core_of * GPC) * GS + pos
    row_of = slot_to_row(core_of, slot_loc).astype(np.int64)

    src = edge_index[0].astype(np.int64)
    dst = edge_index[1].astype(np.int64)

    e_core = [core_of[dst]]
    e_dslot = [slot_loc[dst]]
    e_srow = [row_of[src]]

    # duplicate slots: graph g's pad slots [size_g, GS) copy n0 = starts[g]
    n0_edges = np.where(dst == starts[g_of[dst]])[0]
    n0_g = g_of[dst[n0_edges]]
    max_pad = GS - int(sizes.min())
    for j in range(max_pad):
        gsel_mask = sizes + j < GS
        em = gsel_mask[n0_g]
        gg = n0_g[em]
        pc = gg // GPC
        ps = (gg - pc * GPC) * GS + sizes[gg] + j
        e_core.append(pc)
        e_dslot.append(ps)
        e_srow.append(row_of[src[n0_edges[em]]])

    e_core = np.concatenate(e_core)
    e_dslot = np.concatenate(e_dslot)
    e_srow = np.concatenate(e_srow)

    # per-core, per (group, window) edge cells
    per_core = []
    cnt = np.zeros((C, NG, NW), dtype=np.int64)
    for c in range(C):
        m = e_core == c
        ds, sr = e_dslot[m], e_srow[m]
        g_e = ds // GRP
        w_e = sr >> 15
        # sort by (g, w, srow) for cell grouping + ascending-address locality
        order = np.lexsort((sr, w_e, g_e))
        ds, sr, g_e, w_e = ds[order], sr[order], g_e[order], w_e[order]
        np.add.at(cnt[c], (g_e, w_e), 1)
        per_core.append((ds, sr, g_e, w_e))

    ntile_gw = (cnt.max(axis=0) + 127) // 128  # [NG, NW], shared structure
    tiles_per_group = ntile_gw.sum(axis=1).astype(np.int64)
    assert (tiles_per_group > 0).all()

    # build shared call structure: cohort -> window -> calls of <= CAP_TILES
    cohorts = []
    kt = 0
    icol = 0
    for c0g in range(0, NG, COH):
        groups = list(range(c0g, min(c0g + COH, NG)))
        coh = {"groups": groups, "calls": []}
        for w in range(NW):
            # flat tile list for this (cohort, w)
            tlist = []  # (g,) one entry per k-tile
            for g in groups:
                tlist += [g] * int(ntile_gw[g, w])
            i = 0
            while i < len(tlist):
                chunk = tlist[i : i + CAP_TILES]
                cells = []
                j = 0
                while j < len(chunk):
                    g = chunk[j]
                    n = 1
                    while j + n < len(chunk) and chunk[j + n] == g:
                        n += 1
                    cells.append((g, kt, n))
                    kt += n
                    j += n
                coh["calls"].append((w, icol, cells))
                icol += len(chunk) * 8  # 128 idxs per tile / 16 rows
                i += len(chunk)
        cohorts.append(coh)
    KT = kt
    ICOLS = icol

    # per-core idx16 / relc
    idx16_l, relc_l, degt_l = [], [], []
    for c in range(C):
        ds, sr, g_e, w_e = per_core[c]
        # cell start offsets per (g, w)
        cell_cnt = cnt[c]
        cell_start = np.zeros((NG, NW), dtype=np.int64)
        flat = (g_e * NW + w_e).astype(np.int64)
        # edges sorted by (g, w) already; starts via cumsum over cells
        cc = np.zeros(NG * NW, dtype=np.int64)
        np.add.at(cc, flat, 1)
        cs = np.concatenate([[0], np.cumsum(cc)[:-1]]).reshape(NG, NW)
        cell_start = cs

        idx_lin = np.zeros(KT * 128, dtype=np.int16)
        rel_arr = np.full((128, KT), -1.0, dtype=np.float32)
        consumed = np.zeros((NG, NW), dtype=np.int64)
        for coh in cohorts:
            for (w, icol0, cells) in coh["calls"]:
                for (g, kt0, ntiles) in cells:
                    base = cell_start[g, w]
                    tot = cell_cnt[g, w]
                    for j in range(ntiles):
                        t = kt0 + j
                        lo = consumed[g, w]
                        hi = min(lo + 128, tot)
                        nreal = hi - lo
                        consumed[g, w] = hi
                        sl = slice(base + lo, base + hi)
                        locs = (sr[sl] & (WIN - 1)).astype(np.int16)
                        rels = (ds[sl] % GRP).astype(np.float32)
                        seg = np.zeros(128, dtype=np.int16)
                        seg[:nreal] = locs
                        if nreal > 0 and nreal < 128:
                            seg[nreal:] = locs[-1] if nreal else 0
                        idx_lin[t * 128 : t * 128 + 128] = seg
                        rel_arr[:nreal, t] = rels
        # wrap idx per call block: idx i of call -> (i%16, i//16), replicate x8
        idx16 = np.zeros((128, ICOLS), dtype=np.int16)
        for coh in cohorts:
            for (w, icol0, cells) in coh["calls"]:
                nt = sum(n for (_, _, n) in cells)
                n = nt * 128
                kt0 = cells[0][1]
                blockv = idx_lin[kt0 * 128 : kt0 * 128 + n]
                wrapped = blockv.reshape(n // 16, 16).T  # [16, n/16]
                idx16[:, icol0 : icol0 + n // 16] = np.tile(wrapped, (8, 1))
        idx16_l.append(idx16)
        relc_l.append(np.ascontiguousarray(rel_arr))

        # per-slot degree for the rank-1 BN fold: rows at partitions 0/32/64
        deg_p = np.bincount(ds, minlength=NG * GRP).astype(np.float32)
        deg_p[:SLOTS] += 1.0
        deg_p[SLOTS:] = 0.0
        ncol3 = (NG + 2) // 3
        dg = np.zeros((128, ncol3 * GRP), dtype=np.float32)
        for g in range(NG):
            dg[(g % 3) * 32, (g // 3) * GRP : (g // 3 + 1) * GRP] = deg_p[
                g * GRP : (g + 1) * GRP
            ]
        degt_l.append(dg.astype(np_bf16))

    x_tbl = np.zeros((TBL, 128), dtype=np_bf16)
    x_tbl[row_of, :IN_DIM] = x.astype(np_bf16)

    # per-core feat-major x (incl dup slots), for the SBUF self-add
    xT = []
    xs = x.astype(np.float32)
    for c in range(C):
        xt = np.zeros((128, SHP), dtype=np.float32)
        m = core_of == c
        xt[:IN_DIM, slot_loc[m]] = xs[m].T
        gsel = np.arange(N_GRAPHS)[(np.arange(N_GRAPHS) // GPC) == c]
        for g in gsel:
            sz = sizes[g]
            if sz < GS:
                base = (g - c * GPC) * GS
                xt[:IN_DIM, base + sz : base + GS] = xs[starts[g]][:, None]
        xT.append(xt.astype(np_bf16))

    return HostData(
        GS, GPC, SLOTS, NB, SHP, KT, NW, cohorts, tiles_per_group, ICOLS,
        idx16_l, relc_l, degt_l, x_tbl, cb, xT,
    )


def build_program(hd: HostData):
    """Returns (nc, input_names)."""
    import concourse.bass as bass
    import concourse.mybir as mybir
    import concourse.tile as tile
    from concourse import bacc
    from concourse.masks import make_identity
    from concourse.tile_rust import add_dep_helper

    dt = mybir.dt
    Alu = mybir.AluOpType
    Act = mybir.ActivationFunctionType

    C, D = N_CORES, DIM
    NB, SHP, TBL, NG, KT = hd.nb, hd.shp, hd.tbl, hd.ng, hd.kt_total
    GS, GPC, SLOTS, NW = hd.gs, hd.gpc, hd.slots, hd.nw
    inv_n = 1.0 / (C * SLOTS)

    nc = bacc.Bacc(
        "TRN2", target_bir_lowering=False, debug=False, num_devices=C
    )

    def din(name, shape, dtp=dt.float32):
        return nc.dram_tensor(name, list(shape), dtp, kind="ExternalInput").ap()

    x_tbl_d = din("x_tbl", (TBL, D), dt.bfloat16)
    xT_d = din("xT", (128, SHP), dt.bfloat16)
    idx_d = din("idx", (128, hd.icols), dt.int16)
    rel_d = din("rel", (128, KT))
    ncol3 = (NG + 2) // 3
    degt_d = din("degt", (128, ncol3 * GRP), dt.bfloat16)
    iota_d = din("iota", (128, GRP))
    w1_d = [din(f"w1_{l}", (D, D)) for l in range(3)]
    w2_d = [din(f"w2_{l}", (D, D)) for l in range(3)]
    b1_d = [din(f"b1_{l}", (D, 1)) for l in range(3)]
    b2_d = [din(f"b2_{l}", (D, 1)) for l in range(3)]
    gb_d = din("gb", (D, 6))
    out_d = nc.dram_tensor(
        "pooled", [GPC, 3 * D], dt.float32, kind="ExternalOutput"
    ).ap()

    input_names = (
        ["x_tbl", "xT", "idx", "rel", "degt", "iota"]
        + [f"w1_{l}" for l in range(3)]
        + [f"w2_{l}" for l in range(3)]
        + [f"b1_{l}" for l in range(3)]
        + [f"b2_{l}" for l in range(3)]
        + ["gb"]
    )

    n_pool_chunks = (GPC + 127) // 128
    last_chunk_rows = GPC - (n_pool_chunks - 1) * 128

    with tile.TileContext(nc) as tc:
        with (
            tc.tile_pool(name="const", bufs=1) as cpool,
            tc.tile_pool(name="ebuf", bufs=10) as epool,
            tc.tile_pool(name="spool", bufs=8) as spool,
            tc.tile_pool(name="zin", bufs=2) as zinpool,
            tc.tile_pool(name="zmid", bufs=2) as zmidpool,
            tc.tile_pool(name="rm", bufs=3) as rmpool,
            tc.tile_pool(name="stat", bufs=1) as statpool,
            tc.tile_pool(name="agg_ps", bufs=COH, space="PSUM") as aggpool,
            tc.tile_pool(name="m1_ps", bufs=2, space="PSUM") as m1pool,
            tc.tile_pool(name="m2_ps", bufs=2, space="PSUM") as m2pool,
            tc.tile_pool(name="tr_ps", bufs=1, space="PSUM") as trpool,
            tc.tile_pool(name="dram", bufs=1, space="DRAM") as dpool,
        ):
            # ---- DRAM intermediates ----
            cb = [int(v) for v in hd.cb]
            NCH = N_CHUNKS
            shq = [(cb[k + 1] - cb[k]) * 128 for k in range(NCH)]
            chunk_base = [0]
            for k in range(NCH):
                chunk_base.append(chunk_base[-1] + C * shq[k])
            h_tbl = [
                dpool.tile([TBL, D], dt.bfloat16, name=f"h_{l}", addr_space="Shared")
                for l in range(2)
            ]
            z_ch = [
                dpool.tile([shq[k], D], dt.bfloat16, name=f"z_ch{k}")
                for k in range(NCH)
            ]
            st_in = [
                dpool.tile([D, 2], dt.float32, name=f"st_in{l}") for l in range(3)
            ]
            st_out = [
                dpool.tile([D, 2], dt.float32, name=f"st_out{l}")
                for l in range(3)
            ]

            # ---- constants to SBUF ----
            def load(shape, src_ap, dtp=dt.float32, name=None):
                t = cpool.tile(list(shape), dtp, name=name)
                nc.sync.dma_start(out=t[:], in_=src_ap)
                return t

            idx_sb = load((128, hd.icols), idx_d[:], dt.int16, name="idx_sb")
            rel_sb = load((128, KT), rel_d[:], name="rel_sb")
            degt_sb = load(
                (128, ncol3 * GRP), degt_d[:], dt.bfloat16, name="degt_sb"
            )
            iota_sb = load((128, GRP), iota_d[:], name="iota_sb")
            w1_sb = [load((D, D), w1_d[l][:], name=f"w1sb{l}") for l in range(3)]
            w2_sb = [load((D, D), w2_d[l][:], name=f"w2sb{l}") for l in range(3)]
            b1_sb = [load((D, 1), b1_d[l][:], name=f"b1sb{l}") for l in range(3)]
            b2_sb = [load((D, 1), b2_d[l][:], name=f"b2sb{l}") for l in range(3)]
            gb_sb = load((D, 6), gb_d[:], name="gb_sb")
            w1r0 = cpool.tile([D, D], dt.float32, name="w1r0")
            nc.any.tensor_copy(out=w1r0[:], in_=w1_sb[0][:])
            w2r = []
            for l in range(3):
                t = cpool.tile([D, D], dt.float32, name=f"w2r{l}")
                nc.any.tensor_copy(out=t[:], in_=w2_sb[l][:])
                w2r.append(t)
            ident = cpool.tile([128, 128], dt.bfloat16, name="ident")
            make_identity(nc, ident[:])
            ident32 = cpool.tile([128, 128], dt.float32, name="ident32")
            make_identity(nc, ident32[:])

            s_all = cpool.tile([D, 3], dt.float32, name="s_all")
            t_all = cpool.tile([D, 3], dt.float32, name="t_all")
            w1s_sb = [
                cpool.tile([D, D], dt.float32, name=f"w1s{l}") for l in (1, 2)
            ]
            u_sb = [cpool.tile([1, D], dt.float32, name=f"u{l}") for l in (1, 2)]
            ub_sb = [
                cpool.tile([D, D], dt.bfloat16, name=f"ub{l}") for l in (1, 2)
            ]
            ones_row = cpool.tile([1, D], dt.float32, name="ones_row")
            nc.gpsimd.memset(ones_row[:], 1.0)
            ssum = cpool.tile([128, NG], dt.float32, name="ssum")
            ssq = cpool.tile([128, NG], dt.float32, name="ssq")
            sq_scr = cpool.tile([128, GRP], dt.float32, name="sq_scr")
            stat_scr = cpool.tile([128, 8], dt.float32, name="stat_scr")
            pt_all = [
                cpool.tile([128, GPC], dt.float32, name=f"pt{l}")
                for l in range(3)
            ]
            zkeep = cpool.tile([128, SHP], dt.bfloat16, name="zkeep")
            nc.sync.dma_start(out=zkeep[:], in_=xT_d[:])

            def compute_fold(l):
                st = statpool.tile([D, 2], dt.float32, name="st_ld")
                nc.sync.dma_start(out=st[:], in_=st_out[l][:])
                mu = stat_scr[:, 0:1]
                msq = stat_scr[:, 1:2]
                var = stat_scr[:, 2:3]
                rstd = stat_scr[:, 3:4]
                smu = stat_scr[:, 4:5]
                nc.vector.tensor_scalar_mul(mu, st[:, 0:1], inv_n)
                nc.vector.tensor_scalar_mul(msq, st[:, 1:2], inv_n)
                nc.vector.tensor_tensor(out=var, in0=mu, in1=mu, op=Alu.mult)
                nc.vector.tensor_tensor(
                    out=var, in0=msq, in1=var, op=Alu.subtract
                )
                veps = stat_scr[:, 6:7]
                nc.vector.tensor_scalar_add(veps, var, EPS)
                std = stat_scr[:, 5:6]
                nc.scalar.activation(std, veps, Act.Sqrt)
                nc.vector.reciprocal(rstd, std)
                scol = s_all[:, l : l + 1]
                tcol = t_all[:, l : l + 1]
                nc.vector.tensor_tensor(
                    out=scol, in0=gb_sb[:, 2 * l : 2 * l + 1], in1=rstd,
                    op=Alu.mult,
                )
                nc.vector.tensor_tensor(out=smu, in0=scol, in1=mu, op=Alu.mult)
                nc.vector.tensor_tensor(
                    out=tcol, in0=gb_sb[:, 2 * l + 1 : 2 * l + 2], in1=smu,
                    op=Alu.subtract,
                )
                if l < 2:
                    ln = l + 1
                    nc.vector.tensor_scalar(
                        out=w1s_sb[ln - 1][:], in0=w1_sb[ln][:], scalar1=scol,
                        scalar2=None, op0=Alu.mult,
                    )
                    ups = trpool.tile([1, D], dt.float32, name="ups", tag="tr")
                    nc.tensor.matmul(
                        ups[:], lhsT=tcol, rhs=w1_sb[ln][:], start=True,
                        stop=True,
                    )
                    nc.any.tensor_copy(out=u_sb[ln - 1][:], in_=ups[:])
                    ubp = trpool.tile([D, D], dt.float32, name="ubp", tag="tr")
                    nc.tensor.matmul(
                        ubp[:], lhsT=ones_row[:], rhs=u_sb[ln - 1][:],
                        start=True, stop=True,
                    )
                    nc.any.tensor_copy(out=ub_sb[ln - 1][:], in_=ubp[:])

            def win_ap(tensor_ap, w):
                wl = min(WIN, TBL - w * WIN)
                return tensor_ap[w * WIN : w * WIN + wl, :]

            ag_insts = [[], []]
            for layer in range(3):
                if layer > 0:
                    compute_fold(layer - 1)
                lhs1 = w1r0 if layer == 0 else w1s_sb[layer - 1]
                pt = pt_all[layer]
                tbl_ap = x_tbl_d if layer == 0 else h_tbl[layer - 1][:]

                kt_done = np.zeros(NG, dtype=np.int64)
                first_gather = True
                for coh in hd.cohorts:
                    aggt = {}
                    for (w, icol0, cells) in coh["calls"]:
                        nt = sum(n for (_, _, n) in cells)
                        n = nt * 128
                        et = epool.tile(
                            [128, CAP_TILES * 128], dt.bfloat16, name="ebuf"
                        )
                        gi = nc.gpsimd.dma_gather(
                            et[:, :n].rearrange("p (t f) -> p t f", f=128),
                            win_ap(tbl_ap, w),
                            idx_sb[:, icol0 : icol0 + n // 16],
                            n,
                            n,
                            128,
                        )
                        if first_gather:
                            first_gather = False
                            if layer > 0:
                                for agi in ag_insts[layer - 1]:
                                    add_dep_helper(
                                        getattr(gi, "ins", gi),
                                        getattr(agi, "ins", agi),
                                        reason="gather waits h AllGather",
                                    )
                        col = 0
                        for (g, kt0, ntiles) in cells:
                            W = min(GRP, SHP - g * GRP)
                            if g not in aggt:
                                aggt[g] = aggpool.tile(
                                    [128, GRP], dt.float32, name="agg"
                                )
                            for j in range(ntiles):
                                ktg = kt0 + j
                                s_t = spool.tile(
                                    [128, GRP], dt.bfloat16, name="s_t"
                                )
                                nc.vector.tensor_scalar(
                                    out=s_t[:, :W], in0=iota_sb[:, :W],
                                    scalar1=rel_sb[:, ktg : ktg + 1],
                                    scalar2=None, op0=Alu.is_equal,
                                )
                                nc.tensor.matmul(
                                    aggt[g][:, :W],
                                    lhsT=et[:, col : col + 128],
                                    rhs=s_t[:, :W],
                                    start=(kt_done[g] == 0),
                                    stop=(
                                        kt_done[g] + 1
                                        == hd.tiles_per_group[g]
                                    ),
                                )
                                kt_done[g] += 1
                                col += 128
                    # ---- MLP on the cohort's groups ----
                    for g in coh["groups"]:
                        c0 = g * GRP
                        W = min(GRP, SHP - c0)
                        zin = zinpool.tile([128, GRP], dt.float32, name="zin")
                        nc.vector.tensor_tensor(
                            out=zin[:, :W], in0=aggt[g][:, :W],
                            in1=zkeep[:, c0 : c0 + W], op=Alu.add,
                        )
                        m1 = m1pool.tile([128, GRP], dt.float32, name="m1")
                        nc.tensor.matmul(
                            m1[:, :W], lhsT=lhs1[:], rhs=zin[:, :W],
                            start=True, stop=(layer == 0),
                        )
                        if layer > 0:
                            dp = (g % 3) * 32
                            dc = (g // 3) * GRP
                            nc.tensor.matmul(
                                m1[:, :W],
                                lhsT=ub_sb[layer - 1][dp : dp + 1, :],
                                rhs=degt_sb[dp : dp + 1, dc : dc + W],
                                start=False, stop=True,
                            )
                        z1 = zmidpool.tile([128, GRP], dt.float32, name="z1")
                        nc.scalar.activation(
                            z1[:, :W], m1[:, :W], Act.Relu, bias=b1_sb[layer][:]
                        )
                        m2 = m2pool.tile([128, GRP], dt.float32, name="m2")
                        nc.tensor.matmul(
                            m2[:, :W], lhsT=w2r[layer][:], rhs=z1[:, :W],
                            start=True, stop=True,
                        )
                        z2 = zkeep[:, c0 : c0 + W]
                        wr = min(W, max(0, SLOTS - c0))
                        if wr > 0:
                            nc.scalar.activation(
                                z2[:, :wr], m2[:, :wr], Act.Relu,
                                bias=b2_sb[layer][:],
                                accum_out=ssum[:, g : g + 1],
                            )
                        if wr < W:
                            nc.scalar.activation(
                                z2[:, wr:W], m2[:, wr:W], Act.Relu,
                                bias=b2_sb[layer][:],
                            )
                        if wr > 0:
                            nc.scalar.activation(
                                sq_scr[:, :wr], z2[:, :wr], Act.Square,
                                accum_out=ssq[:, g : g + 1],
                            )
                        # ---- on-the-fly pooling (raw m2; relu+b2 at end) ----
                        pc1 = min(c0 + W, SLOTS)
                        if c0 < pc1:
                            gfirst = (c0 + GS - 1) // GS
                            a = gfirst * GS - c0
                            gend = pc1 // GS
                            nfull = gend - gfirst
                            if nfull > 0:
                                nc.vector.tensor_reduce(
                                    out=pt[:, gfirst:gend],
                                    in_=m2[:, a : a + nfull * GS].rearrange(
                                        "p (g s) -> p g s", s=GS
                                    ),
                                    axis=mybir.AxisListType.X, op=Alu.max,
                                )
                            if a > 0:
                                la = min(a, pc1 - c0)
                                tmpm = stat_scr[:, 7:8]
                                nc.vector.tensor_reduce(
                                    out=tmpm, in_=m2[:, 0:la],
                                    axis=mybir.AxisListType.X, op=Alu.max,
                                )
                                gl = gfirst - 1
                                nc.vector.tensor_tensor(
                                    out=pt[:, gl : gl + 1],
                                    in0=pt[:, gl : gl + 1], in1=tmpm,
                                    op=Alu.max,
                                )
                            r0 = a + max(0, gend - gfirst) * GS
                            if gend >= gfirst and c0 + r0 < pc1:
                                nc.vector.tensor_reduce(
                                    out=pt[:, gend : gend + 1],
                                    in_=m2[:, r0 : pc1 - c0],
                                    axis=mybir.AxisListType.X, op=Alu.max,
                                )
                        # ---- transpose to node-major for the h table ----
                        if layer < 2:
                            for i in range(W // 128):
                                trp = trpool.tile(
                                    [128, 128], dt.bfloat16, name="trp",
                                    tag="tr",
                                )
                                nc.tensor.transpose(
                                    trp[:], z2[:, i * 128 : (i + 1) * 128],
                                    ident[:],
                                )
                                rm = rmpool.tile(
                                    [128, 128], dt.bfloat16, name="rm"
                                )
                                nc.any.tensor_copy(out=rm[:], in_=trp[:])
                                b2i = c0 // 128 + i
                                kch = 0
                                while cb[kch + 1] <= b2i:
                                    kch += 1
                                lr0 = (b2i - cb[kch]) * 128
                                nc.sync.dma_start(
                                    out=z_ch[kch][lr0 : lr0 + 128, :],
                                    in_=rm[:],
                                )
                            # launch chunk AllGather as soon as blocks done
                            for kch in range(NCH):
                                if (cb[kch + 1] - 1) * 128 // GRP == g:
                                    agi = nc.gpsimd.collective_compute(
                                        "AllGather", mybir.AluOpType.bypass,
                                        replica_groups=[list(range(C))],
                                        ins=[z_ch[kch].opt()],
                                        outs=[
                                            h_tbl[layer][
                                                chunk_base[kch] : chunk_base[
                                                    kch
                                                ]
                                                + C * shq[kch],
                                                :,
                                            ].opt()
                                        ],
                                    )
                                    ag_insts[layer].append(agi)

                # ---- stats reduce + AllReduce ----
                sp = statpool.tile([D, 2], dt.float32, name="sp")
                nc.vector.tensor_reduce(
                    out=sp[:, 0:1], in_=ssum[:, :NG],
                    axis=mybir.AxisListType.X, op=Alu.add,
                )
                nc.vector.tensor_reduce(
                    out=sp[:, 1:2], in_=ssq[:, :NG],
                    axis=mybir.AxisListType.X, op=Alu.add,
                )
                nc.sync.dma_start(out=st_in[layer][:], in_=sp[:])
                nc.gpsimd.collective_compute(
                    "AllReduce", mybir.AluOpType.add,
                    replica_groups=[list(range(C))],
                    ins=[st_in[layer].opt()], outs=[st_out[layer].opt()],
                )

            # ---- output: affine + transpose + store ----
            compute_fold(2)
            out_big = cpool.tile(
                [128, n_pool_chunks * 3 * D], dt.float32, name="out_big"
            )
            with tc.tile_pool(name="poolt", bufs=2) as ptpool:
                for l in range(3):
                    pre = ptpool.tile([128, GPC], dt.float32, name="pre")
                    nc.scalar.activation(
                        pre[:], pt_all[l][:], Act.Relu, bias=b2_sb[l][:]
                    )
                    pta = ptpool.tile([128, GPC], dt.float32, name="pta")
                    nc.vector.tensor_scalar(
                        out=pta[:], in0=pre[:],
                        scalar1=s_all[:, l : l + 1],
                        scalar2=t_all[:, l : l + 1], op0=Alu.mult, op1=Alu.add,
                    )
                    for ch in range(n_pool_chunks):
                        rows = (
                            128 if ch < n_pool_chunks - 1 else last_chunk_rows
                        )
                        trp = trpool.tile(
                            [128, 128], dt.float32, name="trpo", tag="tr"
                        )
                        nc.tensor.transpose(
                            trp[:rows, :],
                            pta[:, ch * 128 : ch * 128 + rows], ident32[:],
                        )
                        nc.any.tensor_copy(
                            out=out_big[
                                :rows, ch * 3 * D + l * D : ch * 3 * D
                                + (l + 1) * D
                            ],
                            in_=trp[:rows, :],
                        )
            for ch in range(n_pool_chunks):
                rows = 128 if ch < n_pool_chunks - 1 else last_chunk_rows
                nc.sync.dma_start(
                    out=out_d[ch * 128 : ch * 128 + rows, :],
                    in_=out_big[:rows, ch * 3 * D : (ch + 1) * 3 * D],
                )

    nc.compile()
    return nc, input_names


def make_in_maps(hd: HostData, inputs: dict, input_names):
    iota = np.tile(np.arange(GRP, dtype=np.float32), (128, 1))
    gb = np.zeros((DIM, 6), dtype=np.float32)
    for l in range(3):
        gb[:, 2 * l] = inputs["gamma"][l]
        gb[:, 2 * l + 1] = inputs["beta"][l]
    shared = {
        "x_tbl": hd.x_tbl,
        "iota": np.ascontiguousarray(iota),
        "gb": gb,
    }
    for l in range(3):
        w = np.zeros((DIM, DIM), dtype=np.float32)
        wl = inputs[f"w1_{l}"]
        w[: wl.shape[0], :] = wl
        shared[f"w1_{l}"] = w
        shared[f"w2_{l}"] = np.ascontiguousarray(
            inputs[f"w2_{l}"].astype(np.float32)
        )
        shared[f"b1_{l}"] = inputs[f"b1_{l}"].astype(np.float32).reshape(-1, 1)
        shared[f"b2_{l}"] = inputs[f"b2_{l}"].astype(np.float32).reshape(-1, 1)
    in_maps = []
    for c in range(N_CORES):
        m = dict(shared)
        m["idx"] = hd.idx16[c]
        m["rel"] = hd.relc[c]
        m["degt"] = hd.degt[c]
        m["xT"] = hd.xT[c]
        assert set(m.keys()) == set(input_names)
        in_maps.append(m)
    return in_maps


def _run_sharded_timed(nc, in_maps, n_cores, iters=10, warmup=2):
    """Execute the compiled Bass module via PJRT with device-resident inputs,
    timing `iters` back-to-back dispatches (excludes input upload/compile)."""
    import time

    import jax
    from jax.sharding import Mesh, NamedSharding, PartitionSpec
    from jax.experimental.shard_map import shard_map

    import concourse.mybir as mybir
    from concourse import bass2jax

    bass2jax.install_neuronx_cc_hook()
    partition_name = (
        nc.partition_id_tensor.name if nc.partition_id_tensor else None
    )
    in_names, out_names, out_avals, zero_outs = [], [], [], []
    for alloc in nc.m.functions[0].allocations:
        if not isinstance(alloc, mybir.MemoryLocationSet):
            continue
        name = alloc.memorylocations[0].name
        if alloc.kind == "ExternalInput":
            if name != partition_name:
                in_names.append(name)
        elif alloc.kind == "ExternalOutput":
            out_names.append(name)
            shape = tuple(alloc.tensor_shape)
            dtp = mybir.dt.np(alloc.dtype)
            out_avals.append(jax.core.ShapedArray(shape, dtp))
            zero_outs.append(np.zeros(shape, dtp))
    n_params, n_outs = len(in_names), len(out_avals)
    in_names.extend(out_names)
    if partition_name is not None:
        in_names.append(partition_name)
    donate = tuple(range(n_params, n_params + n_outs))

    def _body(*args):
        operands = list(args)
        if partition_name is not None:
            operands.append(bass2jax.partition_id_tensor())
        outs = bass2jax._bass_exec_p.bind(
            *operands,
            out_avals=tuple(out_avals),
            in_names=tuple(in_names),
            out_names=tuple(out_names),
            lowering_input_output_aliases=(),
            sim_require_finite=True,
            sim_require_nnan=True,
            nc=nc,
        )
        return tuple(outs)

    devices = jax.devices()[:n_cores]
    mesh = Mesh(np.asarray(devices), ("core",))
    pspec = PartitionSpec("core")
    in_specs = (pspec,) * (n_params + n_outs)
    sharded = jax.jit(
        shard_map(
            _body, mesh=mesh, in_specs=in_specs,
            out_specs=(pspec,) * len(out_names), check_rep=False,
        ),
        donate_argnums=donate, keep_unused=True,
    )
    shd = NamedSharding(mesh, pspec)
    per_core = [
        [np.asarray(m[name]) for name in in_names[:n_params]] for m in in_maps
    ]
    dev_in = [
        jax.device_put(
            np.concatenate([per_core[c][i] for c in range(n_cores)], axis=0),
            shd,
        )
        for i in range(n_params)
    ]
    n_calls = warmup + (iters if iters else 0)
    zsets = [
        [
            jax.device_put(
                np.zeros((n_cores * z.shape[0], *z.shape[1:]), z.dtype), shd
            )
            for z in zero_outs
        ]
        for _ in range(max(n_calls, 1))
    ]
    jax.block_until_ready(zsets)
    jax.block_until_ready(dev_in)
    outs = None
    for i in range(warmup):
        outs = sharded(*dev_in, *zsets[i])
        jax.block_until_ready(outs)
    dt = None
    if iters:
        t0 = time.perf_counter()
        ress = [sharded(*dev_in, *zsets[warmup + i]) for i in range(iters)]
        jax.block_until_ready(ress)
        dt = (time.perf_counter() - t0) / iters
        outs = ress[-1]
    if outs is None:
        outs = sharded(*dev_in, *zsets[0])
    results = [
        {
            name: np.asarray(outs[i]).reshape(n_cores, *out_avals[i].shape)[c]
            for i, name in enumerate(out_names)
        }
        for c in range(n_cores)
    ]
    return results, dt


def run(inputs: dict, timed: bool = False):
    x = np.asarray(inputs["x"])
    ei = np.asarray(inputs["edge_index"])
    batch = np.asarray(inputs["batch"])
    hd = prep_host(x, ei, batch)
    nc, input_names = build_program(hd)
    in_maps = make_in_maps(hd, inputs, input_names)
    results, dt = _run_sharded_timed(
        nc, in_maps, N_CORES,
        iters=(200 if timed else 0), warmup=(4 if timed else 1),
    )
    outs = [results[c]["pooled"] for c in range(N_CORES)]
    full = np.concatenate(outs, axis=0).astype(np.float32)
    return full, dt


def kernel(**inputs) -> np.ndarray:
    out, _ = run(inputs, timed=False)
    return out


# revision 18
# speedup vs baseline: 2.5641x; 1.0062x over previous
"""GIN (3-layer) message-passing kernel for Trainium2, 8 NeuronCores.

v2 — batched-gather rewrite of the graph-partition data-parallel design.

  - Graphs assigned to cores by id (750 graphs x 50 nodes per core); nodes
    renumbered into a chunk-interleaved shared h table (4 chunks per layer
    for overlapped AllGathers).  The GIN self term comes from an
    SBUF-resident feat-major copy (zkeep), BN is folded into the next
    layer's first matmul (scale + rank-1 degree correction), stats come
    free from activation accum_out and a 1KB AllReduce.
  - Aggregation (the v2 part): edges are sharded by destination core and
    grouped by (512-slot destination group, 32768-row source window).
    Each (cohort of 3 groups, window) produces one int16 dma_gather call
    (<=1024 indices) instead of per-128-edge indirect DMAs: the SWDGE
    offset walker only supports one offset column per indirect call
    (~1.3us/call measured), while dma_gather moves ~5-6ns/row.  Gathered
    k-tiles (128 edges) are reduced into per-group PSUM banks by one-hot
    matmuls ([128e,128f]^T x [128e,512slots], is_equal-built one-hots).
  - MLP runs in transposed space per 512-col group (fp32r), pooling
    on-the-fly from raw m2 (max commutes with the final monotone affine),
    transposes feed the next layer's node-major h table.
Host assembles the 8 per-core [750, 384] outputs into the full [6000, 384].
"""

import sys

sys.path.insert(0, "/opt/trn_rl_repo")

import math
from dataclasses import dataclass, field

import numpy as np

try:
    from ml_dtypes import bfloat16 as np_bf16
except ImportError:  # pragma: no cover
    import jax.numpy as _jnp

    np_bf16 = _jnp.bfloat16

N_GRAPHS = 6000
N_CORES = 8
IN_DIM = 77
DIM = 128
EPS = 1e-5
GRP = 512  # slots per PSUM aggregation group
COH = 3  # groups per cohort (agg PSUM banks in flight)
NW = 10  # equal-size int16 gather windows (<=32768 rows each)
CAP_TILES = 6  # max k-tiles (128 idxs each) per dma_gather call
N_CHUNKS = 1  # single chunk: Shared DRAM allows one writer inst


@dataclass
class HostData:
    gs: int
    gpc: int
    slots: int
    nb: int
    shp: int
    wsz: int
    npairs: int
    cohorts: list  # per cohort: {'groups': [..], 'calls': [(w, icol0, tn, t0, pairs)]}
    pairs_per_group: np.ndarray  # [NG]
    icols: int
    idx16: list  # per core [128, icols] int16
    relp: list  # per core [128, npairs] f32
    degt: list  # per core [128, GRP] bf16
    x_tbl: np.ndarray  # [TBL, 128] bf16
    cb: np.ndarray  # chunk boundaries in blocks
    xT: list  # per core [128, SHP] bf16

    @property
    def tbl(self):
        return N_CORES * self.shp

    @property
    def ng(self):
        return (self.shp + GRP - 1) // GRP


def prep_host(x: np.ndarray, edge_index: np.ndarray, batch: np.ndarray) -> HostData:
    C = N_CORES
    N = x.shape[0]
    batch = batch.astype(np.int64)
    sizes = np.bincount(batch, minlength=N_GRAPHS)
    assert sizes.min() >= 1
    starts = np.concatenate([[0], np.cumsum(sizes)[:-1]])
    GS = int(sizes.max())
    GPC = N_GRAPHS // C
    SLOTS = GPC * GS
    NB = (SLOTS + 127) // 128
    SHP = NB * 128
    TBL = C * SHP
    NG = (SHP + GRP - 1) // GRP
    WSZ = ((TBL + NW - 1) // NW + 15) // 16 * 16
    assert WSZ <= 32768 and (NW - 1) * WSZ < TBL

    g_of = batch
    pos = np.arange(N, dtype=np.int64) - starts[g_of]
    core_of = g_of // GPC
    slot_loc = (g_of - core_of * GPC) * GS + pos
    row_of = (core_of * SHP + slot_loc).astype(np.int64)

    src = edge_index[0].astype(np.int64)
    dst = edge_index[1].astype(np.int64)

    e_core = [core_of[dst]]
    e_dslot = [slot_loc[dst]]
    e_srow = [row_of[src]]

    # duplicate slots: graph g's pad slots [size_g, GS) copy n0 = starts[g]
    n0_edges = np.where(dst == starts[g_of[dst]])[0]
    n0_g = g_of[dst[n0_edges]]
    max_pad = GS - int(sizes.min())
    for j in range(max_pad):
        gsel_mask = sizes + j < GS
        em = gsel_mask[n0_g]
        gg = n0_g[em]
        pc = gg // GPC
        ps = (gg - pc * GPC) * GS + sizes[gg] + j
        e_core.append(pc)
        e_dslot.append(ps)
        e_srow.append(row_of[src[n0_edges[em]]])

    e_core = np.concatenate(e_core)
    e_dslot = np.concatenate(e_dslot)
    e_srow = np.concatenate(e_srow)

    NCOH = (NG + COH - 1) // COH
    per_core = []
    cnt2 = np.zeros((C, NCOH, NW), dtype=np.int64)
    for c in range(C):
        m = e_core == c
        ds, sr = e_dslot[m], e_srow[m]
        g_e = ds // GRP
        coh_e = g_e // COH
        w_e = sr // WSZ
        order = np.lexsort((sr, g_e, w_e, coh_e))
        ds, sr, g_e, coh_e, w_e = (
            ds[order], sr[order], g_e[order], coh_e[order], w_e[order]
        )
        np.add.at(cnt2[c], (coh_e, w_e), 1)
        per_core.append((ds, sr, g_e))

    npad_cw = ((cnt2.max(axis=0) + 127) // 128) * 128  # [NCOH, NW]

    cell_pos = []
    for c in range(C):
        cc = cnt2[c].reshape(-1)
        cs = np.concatenate([[0], np.cumsum(cc)[:-1]]).reshape(NCOH, NW)
        cell_pos.append(cs)

    cohorts = []
    icol = 0
    pcol = 0
    pairs_per_group = np.zeros(NG, dtype=np.int64)
    for ci in range(NCOH):
        groups = list(range(ci * COH, min((ci + 1) * COH, NG)))
        coh = {"groups": groups, "calls": []}
        for w in range(NW):
            npad = int(npad_cw[ci, w])
            if npad == 0:
                continue
            ntiles = npad // 128
            tile_groups = [set() for _ in range(ntiles)]
            for c in range(C):
                ds, sr, g_e = per_core[c]
                s0 = cell_pos[c][ci, w]
                n_c = int(cnt2[c, ci, w])
                gseq = g_e[s0 : s0 + n_c]
                for t in range(ntiles):
                    lo, hi = t * 128, min((t + 1) * 128, n_c)
                    if lo >= n_c:
                        break
                    for gv in np.unique(gseq[lo:hi]):
                        tile_groups[t].add(int(gv))
            t0 = 0
            while t0 < ntiles:
                tn = min(CAP_TILES, ntiles - t0)
                pairs = []
                for tl in range(tn):
                    for gv in sorted(tile_groups[t0 + tl]):
                        pairs.append((tl, gv, pcol))
                        pairs_per_group[gv] += 1
                        pcol += 1
                coh["calls"].append((w, icol, tn, t0, pairs))
                icol += tn * 8
                t0 += tn
        cohorts.append(coh)
    ICOLS = icol
    NPAIRS = pcol

    idx16_l, relp_l, degt_l = [], [], []
    ncol3 = (NG + 2) // 3
    for c in range(C):
        ds, sr, g_e = per_core[c]
        idx16 = np.zeros((128, ICOLS), dtype=np.int16)
        relp = np.full((128, NPAIRS), -1.0, dtype=np.float32)
        for ci, coh in enumerate(cohorts):
            for (w, icol0, tn, t0, pairs) in coh["calls"]:
                s0 = cell_pos[c][ci, w]
                n_c = int(cnt2[c, ci, w])
                nblk = tn * 128
                base = w * WSZ
                p_lo = t0 * 128
                p_hi = p_lo + nblk
                nreal = max(0, min(p_hi, n_c) - p_lo)
                locs = np.zeros(nblk, dtype=np.int64)
                if nreal > 0:
                    locs[:nreal] = sr[s0 + p_lo : s0 + p_lo + nreal] - base
                    locs[nreal:] = locs[nreal - 1] if nreal else 0
                elif n_c > 0:
                    locs[:] = sr[s0 + n_c - 1] - base
                wrapped = locs.astype(np.int16).reshape(nblk // 16, 16).T
                idx16[:, icol0 : icol0 + nblk // 16] = np.tile(wrapped, (8, 1))
                for (tl, gv, pc_) in pairs:
                    lo = (t0 + tl) * 128
                    hi = min(lo + 128, n_c)
                    if hi <= lo:
                        continue
                    seg_g = g_e[s0 + lo : s0 + hi]
                    seg_d = ds[s0 + lo : s0 + hi]
                    sel = seg_g == gv
                    pp = np.nonzero(sel)[0]
                    relp[pp, pc_] = (seg_d[sel] % GRP).astype(np.float32)
        idx16_l.append(idx16)
        relp_l.append(np.ascontiguousarray(relp))

        # per-slot degree for the rank-1 BN fold: rows at partitions 0/32/64
        deg_p = np.bincount(ds, minlength=NG * GRP).astype(np.float32)
        deg_p[:SLOTS] += 1.0
        deg_p[SLOTS:] = 0.0
        dg = np.zeros((128, ncol3 * GRP), dtype=np.float32)
        for g in range(NG):
            dg[(g % 3) * 32, (g // 3) * GRP : (g // 3 + 1) * GRP] = deg_p[
                g * GRP : (g + 1) * GRP
            ]
        degt_l.append(dg.astype(np_bf16))

    x_tbl = np.zeros((TBL, 128), dtype=np_bf16)
    x_tbl[row_of, :IN_DIM] = x.astype(np_bf16)

    xT = []
    xs = x.astype(np.float32)
    for c in range(C):
        xt = np.zeros((128, SHP), dtype=np.float32)
        m = core_of == c
        xt[:IN_DIM, slot_loc[m]] = xs[m].T
        gsel = np.arange(N_GRAPHS)[(np.arange(N_GRAPHS) // GPC) == c]
        for g in gsel:
            sz = sizes[g]
            if sz < GS:
                base = (g - c * GPC) * GS
                xt[:IN_DIM, base + sz : base + GS] = xs[starts[g]][:, None]
        xT.append(xt.astype(np_bf16))

    return HostData(
        GS, GPC, SLOTS, NB, SHP, WSZ, NPAIRS, cohorts, pairs_per_group, ICOLS,
        idx16_l, relp_l, degt_l, x_tbl, np.array([0, NB]), xT,
    )


def build_program(hd: HostData):
    """Returns (nc, input_names)."""
    import concourse.bass as bass
    import concourse.mybir as mybir
    import concourse.tile as tile
    from concourse import bacc
    from concourse.masks import make_identity
    from concourse.tile_rust import add_dep_helper

    dt = mybir.dt
    Alu = mybir.AluOpType
    Act = mybir.ActivationFunctionType

    C, D = N_CORES, DIM
    NB, SHP, TBL, NG = hd.nb, hd.shp, hd.tbl, hd.ng
    GS, GPC, SLOTS, WSZ = hd.gs, hd.gpc, hd.slots, hd.wsz
    NPAIRS = hd.npairs
    inv_n = 1.0 / (C * SLOTS)

    nc = bacc.Bacc(
        "TRN2", target_bir_lowering=False, debug=False, num_devices=C
    )

    def din(name, shape, dtp=dt.float32):
        return nc.dram_tensor(name, list(shape), dtp, kind="ExternalInput").ap()

    x_tbl_d = din("x_tbl", (TBL, D), dt.bfloat16)
    xT_d = din("xT", (128, SHP), dt.bfloat16)
    idx_d = din("idx", (128, hd.icols), dt.int16)
    rel_d = din("rel", (128, NPAIRS))
    ncol3 = (NG + 2) // 3
    degt_d = din("degt", (128, ncol3 * GRP), dt.bfloat16)
    iota_d = din("iota", (128, GRP))
    w1_d = [din(f"w1_{l}", (D, D)) for l in range(3)]
    w2_d = [din(f"w2_{l}", (D, D)) for l in range(3)]
    b1_d = [din(f"b1_{l}", (D, 1)) for l in range(3)]
    b2_d = [din(f"b2_{l}", (D, 1)) for l in range(3)]
    gb_d = din("gb", (D, 6))
    out_d = nc.dram_tensor(
        "pooled", [GPC, 3 * D], dt.float32, kind="ExternalOutput"
    ).ap()

    input_names = (
        ["x_tbl", "xT", "idx", "rel", "degt", "iota"]
        + [f"w1_{l}" for l in range(3)]
        + [f"w2_{l}" for l in range(3)]
        + [f"b1_{l}" for l in range(3)]
        + [f"b2_{l}" for l in range(3)]
        + ["gb"]
    )

    n_pool_chunks = (GPC + 127) // 128
    last_chunk_rows = GPC - (n_pool_chunks - 1) * 128

    with tile.TileContext(nc) as tc:
        with (
            tc.tile_pool(name="const", bufs=1) as cpool,
            tc.tile_pool(name="ebuf", bufs=10) as epool,
            tc.tile_pool(name="spool", bufs=8) as spool,
            tc.tile_pool(name="zin", bufs=2) as zinpool,
            tc.tile_pool(name="zmid", bufs=2) as zmidpool,
            tc.tile_pool(name="rm", bufs=3) as rmpool,
            tc.tile_pool(name="stat", bufs=1) as statpool,
            tc.tile_pool(name="agg_ps", bufs=COH, space="PSUM") as aggpool,
            tc.tile_pool(name="m1_ps", bufs=2, space="PSUM") as m1pool,
            tc.tile_pool(name="m2_ps", bufs=2, space="PSUM") as m2pool,
            tc.tile_pool(name="tr_ps", bufs=1, space="PSUM") as trpool,
            tc.tile_pool(name="dram", bufs=1, space="DRAM") as dpool,
        ):
            # ---- DRAM intermediates ----
            cb = [int(v) for v in hd.cb]
            NCH = N_CHUNKS
            shq = [(cb[k + 1] - cb[k]) * 128 for k in range(NCH)]
            chunk_base = [0]
            for k in range(NCH):
                chunk_base.append(chunk_base[-1] + C * shq[k])
            h_tbl = [
                dpool.tile([TBL, D], dt.bfloat16, name=f"h_{l}", addr_space="Shared")
                for l in range(2)
            ]
            z_ch = [
                dpool.tile([shq[k], D], dt.bfloat16, name=f"z_ch{k}")
                for k in range(NCH)
            ]
            st_in = [
                dpool.tile([D, 2], dt.float32, name=f"st_in{l}") for l in range(3)
            ]
            st_out = [
                dpool.tile([D, 2], dt.float32, name=f"st_out{l}")
                for l in range(3)
            ]

            # ---- constants to SBUF ----
            def load(shape, src_ap, dtp=dt.float32, name=None):
                t = cpool.tile(list(shape), dtp, name=name)
                nc.sync.dma_start(out=t[:], in_=src_ap)
                return t

            idx_sb = load((128, hd.icols), idx_d[:], dt.int16, name="idx_sb")
            rel_sb = load((128, NPAIRS), rel_d[:], name="rel_sb")
            degt_sb = load(
                (128, ncol3 * GRP), degt_d[:], dt.bfloat16, name="degt_sb"
            )
            iota_sb = load((128, GRP), iota_d[:], name="iota_sb")
            w1_sb = [load((D, D), w1_d[l][:], name=f"w1sb{l}") for l in range(3)]
            w2_sb = [load((D, D), w2_d[l][:], name=f"w2sb{l}") for l in range(3)]
            b1_sb = [load((D, 1), b1_d[l][:], name=f"b1sb{l}") for l in range(3)]
            b2_sb = [load((D, 1), b2_d[l][:], name=f"b2sb{l}") for l in range(3)]
            gb_sb = load((D, 6), gb_d[:], name="gb_sb")
            w1r0 = cpool.tile([D, D], dt.float32, name="w1r0")
            nc.any.tensor_copy(out=w1r0[:], in_=w1_sb[0][:])
            w2r = []
            for l in range(3):
                t = cpool.tile([D, D], dt.float32, name=f"w2r{l}")
                nc.any.tensor_copy(out=t[:], in_=w2_sb[l][:])
                w2r.append(t)
            ident = cpool.tile([128, 128], dt.bfloat16, name="ident")
            make_identity(nc, ident[:])
            ident32 = cpool.tile([128, 128], dt.float32, name="ident32")
            make_identity(nc, ident32[:])

            s_all = cpool.tile([D, 3], dt.float32, name="s_all")
            t_all = cpool.tile([D, 3], dt.float32, name="t_all")
            w1s_sb = [
                cpool.tile([D, D], dt.float32, name=f"w1s{l}") for l in (1, 2)
            ]
            u_sb = [cpool.tile([1, D], dt.float32, name=f"u{l}") for l in (1, 2)]
            ub_sb = [
                cpool.tile([D, D], dt.bfloat16, name=f"ub{l}") for l in (1, 2)
            ]
            ones_row = cpool.tile([1, D], dt.float32, name="ones_row")
            nc.gpsimd.memset(ones_row[:], 1.0)
            ssum = cpool.tile([128, NG], dt.float32, name="ssum")
            ssq = cpool.tile([128, NG], dt.float32, name="ssq")
            sq_scr = cpool.tile([128, GRP], dt.float32, name="sq_scr")
            stat_scr = cpool.tile([128, 8], dt.float32, name="stat_scr")
            pt_all = [
                cpool.tile([128, GPC], dt.float32, name=f"pt{l}")
                for l in range(3)
            ]
            zkeep = cpool.tile([128, SHP], dt.bfloat16, name="zkeep")
            nc.sync.dma_start(out=zkeep[:], in_=xT_d[:])

            def compute_fold(l):
                st = statpool.tile([D, 2], dt.float32, name="st_ld")
                nc.sync.dma_start(out=st[:], in_=st_out[l][:])
                mu = stat_scr[:, 0:1]
                msq = stat_scr[:, 1:2]
                var = stat_scr[:, 2:3]
                rstd = stat_scr[:, 3:4]
                smu = stat_scr[:, 4:5]
                nc.vector.tensor_scalar_mul(mu, st[:, 0:1], inv_n)
                nc.vector.tensor_scalar_mul(msq, st[:, 1:2], inv_n)
                nc.vector.tensor_tensor(out=var, in0=mu, in1=mu, op=Alu.mult)
                nc.vector.tensor_tensor(
                    out=var, in0=msq, in1=var, op=Alu.subtract
                )
                veps = stat_scr[:, 6:7]
                nc.vector.tensor_scalar_add(veps, var, EPS)
                std = stat_scr[:, 5:6]
                nc.scalar.activation(std, veps, Act.Sqrt)
                nc.vector.reciprocal(rstd, std)
                scol = s_all[:, l : l + 1]
                tcol = t_all[:, l : l + 1]
                nc.vector.tensor_tensor(
                    out=scol, in0=gb_sb[:, 2 * l : 2 * l + 1], in1=rstd,
                    op=Alu.mult,
                )
                nc.vector.tensor_tensor(out=smu, in0=scol, in1=mu, op=Alu.mult)
                nc.vector.tensor_tensor(
                    out=tcol, in0=gb_sb[:, 2 * l + 1 : 2 * l + 2], in1=smu,
                    op=Alu.subtract,
                )
                if l < 2:
                    ln = l + 1
                    nc.vector.tensor_scalar(
                        out=w1s_sb[ln - 1][:], in0=w1_sb[ln][:], scalar1=scol,
                        scalar2=None, op0=Alu.mult,
                    )
                    ups = trpool.tile([1, D], dt.float32, name="ups", tag="tr")
                    nc.tensor.matmul(
                        ups[:], lhsT=tcol, rhs=w1_sb[ln][:], start=True,
                        stop=True,
                    )
                    nc.any.tensor_copy(out=u_sb[ln - 1][:], in_=ups[:])
                    ubp = trpool.tile([D, D], dt.float32, name="ubp", tag="tr")
                    nc.tensor.matmul(
                        ubp[:], lhsT=ones_row[:], rhs=u_sb[ln - 1][:],
                        start=True, stop=True,
                    )
                    nc.any.tensor_copy(out=ub_sb[ln - 1][:], in_=ubp[:])

            def win_ap(tensor_ap, w):
                wl = min(WSZ, TBL - w * WSZ)
                return tensor_ap[w * WSZ : w * WSZ + wl, :]

            ag_insts = [[], []]
            for layer in range(3):
                if layer > 0:
                    compute_fold(layer - 1)
                lhs1 = w1r0 if layer == 0 else w1s_sb[layer - 1]
                pt = pt_all[layer]
                tbl_ap = x_tbl_d if layer == 0 else h_tbl[layer - 1][:]

                pr_done = np.zeros(NG, dtype=np.int64)
                first_gather = True
                for coh in hd.cohorts:
                    aggt = {}
                    for (w, icol0, tn, t0, pairs) in coh["calls"]:
                        n = tn * 128
                        et = epool.tile(
                            [128, CAP_TILES * 128], dt.bfloat16, name="ebuf"
                        )
                        gi = nc.gpsimd.dma_gather(
                            et[:, :n].rearrange("p (t f) -> p t f", f=128),
                            win_ap(tbl_ap, w),
                            idx_sb[:, icol0 : icol0 + n // 16],
                            n,
                            n,
                            128,
                        )
                        if first_gather:
                            first_gather = False
                            if layer > 0:
                                for agi in ag_insts[layer - 1]:
                                    add_dep_helper(
                                        getattr(gi, "ins", gi),
                                        getattr(agi, "ins", agi),
                                        reason="gather waits h AllGather",
                                    )
                        for (tl, g, pc_) in pairs:
                            W = min(GRP, SHP - g * GRP)
                            if g not in aggt:
                                aggt[g] = aggpool.tile(
                                    [128, GRP], dt.float32, name="agg"
                                )
                            s_t = spool.tile(
                                [128, GRP], dt.bfloat16, name="s_t"
                            )
                            nc.vector.tensor_scalar(
                                out=s_t[:, :W], in0=iota_sb[:, :W],
                                scalar1=rel_sb[:, pc_ : pc_ + 1],
                                scalar2=None, op0=Alu.is_equal,
                            )
                            nc.tensor.matmul(
                                aggt[g][:, :W],
                                lhsT=et[:, tl * 128 : (tl + 1) * 128],
                                rhs=s_t[:, :W],
                                start=(pr_done[g] == 0),
                                stop=(
                                    pr_done[g] + 1 == hd.pairs_per_group[g]
                                ),
                            )
                            pr_done[g] += 1
                    # ---- MLP on the cohort's groups ----
                    for g in coh["groups"]:
                        c0 = g * GRP
                        W = min(GRP, SHP - c0)
                        zin = zinpool.tile([128, GRP], dt.float32, name="zin")
                        nc.vector.tensor_tensor(
                            out=zin[:, :W], in0=aggt[g][:, :W],
                            in1=zkeep[:, c0 : c0 + W], op=Alu.add,
                        )
                        m1 = m1pool.tile([128, GRP], dt.float32, name="m1")
                        nc.tensor.matmul(
                            m1[:, :W], lhsT=lhs1[:], rhs=zin[:, :W],
                            start=True, stop=(layer == 0),
                        )
                        if layer > 0:
                            dp = (g % 3) * 32
                            dc = (g // 3) * GRP
                            nc.tensor.matmul(
                                m1[:, :W],
                                lhsT=ub_sb[layer - 1][dp : dp + 1, :],
                                rhs=degt_sb[dp : dp + 1, dc : dc + W],
                                start=False, stop=True,
                            )
                        z1 = zmidpool.tile([128, GRP], dt.float32, name="z1")
                        nc.scalar.activation(
                            z1[:, :W], m1[:, :W], Act.Relu, bias=b1_sb[layer][:]
                        )
                        m2 = m2pool.tile([128, GRP], dt.float32, name="m2")
                        nc.tensor.matmul(
                            m2[:, :W], lhsT=w2r[layer][:], rhs=z1[:, :W],
                            start=True, stop=True,
                        )
                        z2 = zkeep[:, c0 : c0 + W]
                        wr = min(W, max(0, SLOTS - c0))
                        if wr > 0:
                            nc.scalar.activation(
                                z2[:, :wr], m2[:, :wr], Act.Relu,
                                bias=b2_sb[layer][:],
                                accum_out=ssum[:, g : g + 1],
                            )
                        if wr < W:
                            nc.scalar.activation(
                                z2[:, wr:W], m2[:, wr:W], Act.Relu,
                                bias=b2_sb[layer][:],
                            )
                        if wr > 0:
                            nc.scalar.activation(
                                sq_scr[:, :wr], z2[:, :wr], Act.Square,
                                accum_out=ssq[:, g : g + 1],
                            )
                        # ---- on-the-fly pooling (raw m2; relu+b2 at end) ----
                        pc1 = min(c0 + W, SLOTS)
                        if c0 < pc1:
                            gfirst = (c0 + GS - 1) // GS
                            a = gfirst * GS - c0
                            gend = pc1 // GS
                            nfull = gend - gfirst
                            if nfull > 0:
                                nc.vector.tensor_reduce(
                                    out=pt[:, gfirst:gend],
                                    in_=m2[:, a : a + nfull * GS].rearrange(
                                        "p (g s) -> p g s", s=GS
                                    ),
                                    axis=mybir.AxisListType.X, op=Alu.max,
                                )
                            if a > 0:
                                la = min(a, pc1 - c0)
                                tmpm = stat_scr[:, 7:8]
                                nc.vector.tensor_reduce(
                                    out=tmpm, in_=m2[:, 0:la],
                                    axis=mybir.AxisListType.X, op=Alu.max,
                                )
                                gl = gfirst - 1
                                nc.vector.tensor_tensor(
                                    out=pt[:, gl : gl + 1],
                                    in0=pt[:, gl : gl + 1], in1=tmpm,
                                    op=Alu.max,
                                )
                            r0 = a + max(0, gend - gfirst) * GS
                            if gend >= gfirst and c0 + r0 < pc1:
                                nc.vector.tensor_reduce(
                                    out=pt[:, gend : gend + 1],
                                    in_=m2[:, r0 : pc1 - c0],
                                    axis=mybir.AxisListType.X, op=Alu.max,
                                )
                        # ---- transpose to node-major for the h table ----
                        if layer < 2:
                            for i in range(W // 128):
                                trp = trpool.tile(
                                    [128, 128], dt.bfloat16, name="trp",
                                    tag="tr",
                                )
                                nc.tensor.transpose(
                                    trp[:], z2[:, i * 128 : (i + 1) * 128],
                                    ident[:],
                                )
                                rm = rmpool.tile(
                                    [128, 128], dt.bfloat16, name="rm"
                                )
                                nc.any.tensor_copy(out=rm[:], in_=trp[:])
                                b2i = c0 // 128 + i
                                kch = 0
                                while cb[kch + 1] <= b2i:
                                    kch += 1
                                lr0 = (b2i - cb[kch]) * 128
                                nc.sync.dma_start(
                                    out=z_ch[kch][lr0 : lr0 + 128, :],
                                    in_=rm[:],
                                )
                            # launch chunk AllGather as soon as blocks done
                            for kch in range(NCH):
                                if (cb[kch + 1] - 1) * 128 // GRP == g:
                                    agi = nc.gpsimd.collective_compute(
                                        "AllGather", mybir.AluOpType.bypass,
                                        replica_groups=[list(range(C))],
                                        ins=[z_ch[kch].opt()],
                                        outs=[
                                            h_tbl[layer][
                                                chunk_base[kch] : chunk_base[
                                                    kch
                                                ]
                                                + C * shq[kch],
                                                :,
                                            ].opt()
                                        ],
                                    )
                                    ag_insts[layer].append(agi)

                # ---- stats reduce + AllReduce ----
                sp = statpool.tile([D, 2], dt.float32, name="sp")
                nc.vector.tensor_reduce(
                    out=sp[:, 0:1], in_=ssum[:, :NG],
                    axis=mybir.AxisListType.X, op=Alu.add,
                )
                nc.vector.tensor_reduce(
                    out=sp[:, 1:2], in_=ssq[:, :NG],
                    axis=mybir.AxisListType.X, op=Alu.add,
                )
                nc.sync.dma_start(out=st_in[layer][:], in_=sp[:])
                nc.gpsimd.collective_compute(
                    "AllReduce", mybir.AluOpType.add,
                    replica_groups=[list(range(C))],
                    ins=[st_in[layer].opt()], outs=[st_out[layer].opt()],
                )

            # ---- output: affine + transpose + store ----
            compute_fold(2)
            out_big = cpool.tile(
                [128, n_pool_chunks * 3 * D], dt.float32, name="out_big"
            )
            with tc.tile_pool(name="poolt", bufs=2) as ptpool:
                for l in range(3):
                    pre = ptpool.tile([128, GPC], dt.float32, name="pre")
                    nc.scalar.activation(
                        pre[:], pt_all[l][:], Act.Relu, bias=b2_sb[l][:]
                    )
                    pta = ptpool.tile([128, GPC], dt.float32, name="pta")
                    nc.vector.tensor_scalar(
                        out=pta[:], in0=pre[:],
                        scalar1=s_all[:, l : l + 1],
                        scalar2=t_all[:, l : l + 1], op0=Alu.mult, op1=Alu.add,
                    )
                    for ch in range(n_pool_chunks):
                        rows = (
                            128 if ch < n_pool_chunks - 1 else last_chunk_rows
                        )
                        trp = trpool.tile(
                            [128, 128], dt.float32, name="trpo", tag="tr"
                        )
                        nc.tensor.transpose(
                            trp[:rows, :],
                            pta[:, ch * 128 : ch * 128 + rows], ident32[:],
                        )
                        nc.any.tensor_copy(
                            out=out_big[
                                :rows, ch * 3 * D + l * D : ch * 3 * D
                                + (l + 1) * D
                            ],
                            in_=trp[:rows, :],
                        )
            for ch in range(n_pool_chunks):
                rows = 128 if ch < n_pool_chunks - 1 else last_chunk_rows
                nc.sync.dma_start(
                    out=out_d[ch * 128 : ch * 128 + rows, :],
                    in_=out_big[:rows, ch * 3 * D : (ch + 1) * 3 * D],
                )

    nc.compile()
    return nc, input_names


def make_in_maps(hd: HostData, inputs: dict, input_names):
    iota = np.tile(np.arange(GRP, dtype=np.float32), (128, 1))
    gb = np.zeros((DIM, 6), dtype=np.float32)
    for l in range(3):
        gb[:, 2 * l] = inputs["gamma"][l]
        gb[:, 2 * l + 1] = inputs["beta"][l]
    shared = {
        "x_tbl": hd.x_tbl,
        "iota": np.ascontiguousarray(iota),
        "gb": gb,
    }
    for l in range(3):
        w = np.zeros((DIM, DIM), dtype=np.float32)
        wl = inputs[f"w1_{l}"]
        w[: wl.shape[0], :] = wl
        shared[f"w1_{l}"] = w
        shared[f"w2_{l}"] = np.ascontiguousarray(
            inputs[f"w2_{l}"].astype(np.float32)
        )
        shared[f"b1_{l}"] = inputs[f"b1_{l}"].astype(np.float32).reshape(-1, 1)
        shared[f"b2_{l}"] = inputs[f"b2_{l}"].astype(np.float32).reshape(-1, 1)
    in_maps = []
    for c in range(N_CORES):
        m = dict(shared)
        m["idx"] = hd.idx16[c]
        m["rel"] = hd.relp[c]
        m["degt"] = hd.degt[c]
        m["xT"] = hd.xT[c]
        assert set(m.keys()) == set(input_names)
        in_maps.append(m)
    return in_maps


def _run_sharded_timed(nc, in_maps, n_cores, iters=10, warmup=2):
    """Execute the compiled Bass module via PJRT with device-resident inputs,
    timing `iters` back-to-back dispatches (excludes input upload/compile)."""
    import time

    import jax
    from jax.sharding import Mesh, NamedSharding, PartitionSpec
    from jax.experimental.shard_map import shard_map

    import concourse.mybir as mybir
    from concourse import bass2jax

    bass2jax.install_neuronx_cc_hook()
    partition_name = (
        nc.partition_id_tensor.name if nc.partition_id_tensor else None
    )
    in_names, out_names, out_avals, zero_outs = [], [], [], []
    for alloc in nc.m.functions[0].allocations:
        if not isinstance(alloc, mybir.MemoryLocationSet):
            continue
        name = alloc.memorylocations[0].name
        if alloc.kind == "ExternalInput":
            if name != partition_name:
                in_names.append(name)
        elif alloc.kind == "ExternalOutput":
            out_names.append(name)
            shape = tuple(alloc.tensor_shape)
            dtp = mybir.dt.np(alloc.dtype)
            out_avals.append(jax.core.ShapedArray(shape, dtp))
            zero_outs.append(np.zeros(shape, dtp))
    n_params, n_outs = len(in_names), len(out_avals)
    in_names.extend(out_names)
    if partition_name is not None:
        in_names.append(partition_name)
    donate = tuple(range(n_params, n_params + n_outs))

    def _body(*args):
        operands = list(args)
        if partition_name is not None:
            operands.append(bass2jax.partition_id_tensor())
        outs = bass2jax._bass_exec_p.bind(
            *operands,
            out_avals=tuple(out_avals),
            in_names=tuple(in_names),
            out_names=tuple(out_names),
            lowering_input_output_aliases=(),
            sim_require_finite=True,
            sim_require_nnan=True,
            nc=nc,
        )
        return tuple(outs)

    devices = jax.devices()[:n_cores]
    mesh = Mesh(np.asarray(devices), ("core",))
    pspec = PartitionSpec("core")
    in_specs = (pspec,) * (n_params + n_outs)
    sharded = jax.jit(
        shard_map(
            _body, mesh=mesh, in_specs=in_specs,
            out_specs=(pspec,) * len(out_names), check_rep=False,
        ),
        donate_argnums=donate, keep_unused=True,
    )
    shd = NamedSharding(mesh, pspec)
    per_core = [
        [np.asarray(m[name]) for name in in_names[:n_params]] for m in in_maps
    ]
    dev_in = [
        jax.device_put(
            np.concatenate([per_core[c][i] for c in range(n_cores)], axis=0),
            shd,
        )
        for i in range(n_params)
    ]
    n_calls = warmup + (iters if iters else 0)
    zsets = [
        [
            jax.device_put(
                np.zeros((n_cores * z.shape[0], *z.shape[1:]), z.dtype), shd
            )
            for z in zero_outs
        ]
        for _ in range(max(n_calls, 1))
    ]
    jax.block_until_ready(zsets)
    jax.block_until_ready(dev_in)
    outs = None
    for i in range(warmup):
        outs = sharded(*dev_in, *zsets[i])
        jax.block_until_ready(outs)
    dt = None
    if iters:
        t0 = time.perf_counter()
        ress = [sharded(*dev_in, *zsets[warmup + i]) for i in range(iters)]
        jax.block_until_ready(ress)
        dt = (time.perf_counter() - t0) / iters
        outs = ress[-1]
    if outs is None:
        outs = sharded(*dev_in, *zsets[0])
    results = [
        {
            name: np.asarray(outs[i]).reshape(n_cores, *out_avals[i].shape)[c]
            for i, name in enumerate(out_names)
        }
        for c in range(n_cores)
    ]
    return results, dt


def run(inputs: dict, timed: bool = False):
    x = np.asarray(inputs["x"])
    ei = np.asarray(inputs["edge_index"])
    batch = np.asarray(inputs["batch"])
    hd = prep_host(x, ei, batch)
    nc, input_names = build_program(hd)
    in_maps = make_in_maps(hd, inputs, input_names)
    results, dt = _run_sharded_timed(
        nc, in_maps, N_CORES,
        iters=(200 if timed else 0), warmup=(4 if timed else 1),
    )
    outs = [results[c]["pooled"] for c in range(N_CORES)]
    full = np.concatenate(outs, axis=0).astype(np.float32)
    return full, dt


def kernel(**inputs) -> np.ndarray:
    out, _ = run(inputs, timed=False)
    return out
